# revision 19
# baseline (speedup 1.0000x reference)
"""Trainium2 Bass kernel for the aux-attention module.

reference (per batch b):
    inputs = concat([enc[b], broadcast(hs[b])], -1)          # (S, 4096)
    hidden = tanh(inputs @ W1.T + b1)                        # (S, 1024)
    e      = hidden @ w2.T                                   # (S,)
    alpha  = softmax(e)
    ctx    = alpha @ enc[b]                                  # (3072,)
    out[b] = ctx @ W3.T + b3                                 # (1024,)

Strategy: data-parallel over batch (4 batches/core x 8 cores), weights
replicated. PE matmuls in fp16 (fp32 PSUM). Softmax without max-subtraction:
w = exp(e - 4) unnormalized, 1/sum(w) folded into the final output scaling.

The hs@W1h + b1 per-batch bias rows (hb) are computed on HOST (tiny GEMV) and
shipped 128-replicated so the per-tile bias add runs on DVE (in-place into
PSUM) instead of costing PE outer-product matmuls. Tiles are processed
j-major (batch rotates fastest) so the four final ctx chains pipeline under
each other's matmul streams. The first two tiles' k-loops are interleaved so
early PE demand for W1 chunks (~6.3 MB) matches DMA supply.

Per core, per 128-row tile (single pass over enc, f-major layout from host):
  - hidden = tanh(enc_tile @ W1e.T + hb) : PE k-loop + DVE bias add + ACT
  - e column via one fused DVE multiply+accumulate against broadcast w2
  - w = exp(e-4) (ACT), column -> row via a tiny cross-partition DMA,
    broadcast across partitions (K=1 matmul outer product), then
    ctx_partial[f-chunk] = sum_s w[s]*enc[f, s] as a DVE multiply +
    per-chunk reduce on the same f-major tile already in SBUF. This chain is
    emitted one row-tile behind the matmul stream so the PE never waits.
Tail: inv_l via reduce+reciprocal, out = (ctxT @ W3.T) * inv_l + b3.
"""

import numpy as np

try:  # persistent compile cache: repeated runs skip the walrus compile
    import jax

    jax.config.update("jax_compilation_cache_dir", "/tmp/jax_neff_cache")
    jax.config.update("jax_persistent_cache_min_compile_time_secs", 1.0)
except Exception:
    pass

import concourse.bass as bass
import concourse.tile as tile
from concourse import mybir
from concourse.bass import ds
from concourse import bass_utils

# ---------------------------------------------------------------------------
# Walrus in this container caps sync waits per instruction (one; two for
# EventSemaphore). Tile's tail drain carries one wait per live semaphore and
# Tile occasionally leaks multi-wait instructions; split extras onto cheap
# carriers.
from concourse import tile as _tile_mod
from concourse import mybir as _mybir


def _patched_drain_and_barrier(self, tick_clock, wait_clock):
    nc = self.nc
    drain_inst = nc.sync.drain()
    wait_clock.add_sem_waits(
        drain_inst.ins, _tile_mod.ScopedClock({None: tick_clock.global_clock})
    )
    si = drain_inst.ins.sync_info
    waits = list(si.on_wait) if si is not None else []
    if len(waits) > 1:
        drain_inst.ins.sync_info = _mybir.SyncInfo(on_update=[], on_wait=waits[:1])
        for w in waits[1:]:
            extra = nc.sync.nop(nofuse=True, hint="drain_wait_split")
            extra.ins.sync_info = _mybir.SyncInfo(on_update=[], on_wait=[w])
    nc.all_engine_barrier()
    assert self.sems is not None
    popped = nc._tile_sem_poison_stack.pop()
    assert popped is self._sem_poison
    nc.clear_and_free_semaphores(list(self.sems.allocated().values()))
    nc.all_engine_barrier()


_tile_mod.TileContext._drain_and_barrier = _patched_drain_and_barrier


def _split_multiwaits(nc):
    for fn in nc.m.functions:
        for blk in fn.blocks:
            out, changed = [], False
            for inst in list(blk.instructions):
                si = inst.sync_info
                waits = list(si.on_wait) if si is not None else []
                cap = 2 if inst.opcode == "EventSemaphore" else 1
                if len(waits) > cap:
                    changed = True
                    for idx, w in enumerate(waits[:-cap]):
                        nop = _mybir.InstNoOp(
                            name=f"{inst.name}-wsplit{idx}", ins=[], outs=[]
                        )
                        nop.engine = inst.engine
                        nop.sync_info = _mybir.SyncInfo(on_update=[], on_wait=[w])
                        out.append(nop)
                    inst.sync_info = _mybir.SyncInfo(
                        on_update=list(si.on_update), on_wait=waits[-cap:]
                    )
                out.append(inst)
            if changed:
                blk.instructions = out


# ---------------------------------------------------------------------------

F16 = mybir.dt.float16
F32 = mybir.dt.float32

N_CORES = 8
B, S, DIM, F = 32, 1024, 1024, 3072  # F = enc feature dim; DIM = model dim
KF = F // 128  # 24 enc k-tiles
EXP_SHIFT = -4.0  # w = exp(e + EXP_SHIFT); e is O(1), shift keeps fp16 safe


def _bcast_free(ap, n, at=1):
    """Insert a step-0 (broadcast) free dim of size n at position `at`."""
    aps = list(ap.ap)
    aps.insert(at, [0, n])
    return bass.AP(tensor=ap.tensor, offset=ap.offset, ap=aps)


def _bcast_part(ap, p):
    """View a [1, n] row as a [1, p, n] replication via a step-0 middle dim
    (DMA source view; the dest AP's partition dim carries the fan-out)."""
    aps = list(ap.ap)
    assert aps[0][1] == 1
    return bass.AP(
        tensor=ap.tensor, offset=ap.offset, ap=[aps[0], [0, p]] + aps[1:]
    )


def build_bass(nb, j_tiles):
    """nb batches per core, j_tiles row-tiles of 128 per batch."""
    nj = nb * j_tiles
    nc = bass.Bass()
    encT = nc.declare_dram_parameter("encT", [nj, 128, KF, 128], F16, isOutput=False)
    w1t = nc.declare_dram_parameter("w1t", [128, KF, DIM], F16, isOutput=False)
    w3t = nc.declare_dram_parameter("w3t", [KF, 128, DIM], F16, isOutput=False)
    hbb = nc.declare_dram_parameter("hbb", [128, nb, DIM], F16, isOutput=False)
    w2b = nc.declare_dram_parameter("w2b", [128, DIM], F16, isOutput=False)
    b3b = nc.declare_dram_parameter("b3b", [nb, DIM], F32, isOutput=False)
    onesb = nc.declare_dram_parameter("onesb", [1, 128], F16, isOutput=False)
    out_d = nc.declare_dram_parameter("out", [nb, DIM], F32, isOutput=True)

    # tile t -> (b, j), j-major: the last nb tiles are the final row-tile of
    # each batch, so their ctx chains pipeline under each other's matmuls.
    seq = [(t % nb, t // nb) for t in range(nj)]
    # first tiles run interleaved k-loops so PE demand for each W1 chunk
    # tracks DMA supply; with 3 tiles the third lags by `lag` k-steps (its
    # PSUM group accumulates k=lag..KF-1 first, then wraps to k=0..lag-1)
    n_skew = 3 if nj >= 4 else min(2, nj)

    with tile.TileContext(nc) as tc:
        with (
            tc.tile_pool(name="consts", bufs=1) as consts,
            tc.tile_pool(name="encT", bufs=5) as encT_pool,
            tc.tile_pool(name="tanh", bufs=3) as tanh_pool,
            tc.tile_pool(name="scratch", bufs=1) as scratch_pool,
            tc.tile_pool(name="prod", bufs=2) as prod_pool,
            tc.tile_pool(name="wrow", bufs=3) as wrow_pool,
            tc.tile_pool(name="ctxa", bufs=2) as ctxa_pool,
            tc.tile_pool(name="ps", bufs=4, space="PSUM") as ps,
        ):
            # ---- resident constants ----
            # DMA emission order is the schedule priority: the PE needs et
            # tiles for the skew group plus W1 chunks in k order, everything
            # else after.
            w1t_sb = consts.tile([128, KF, DIM], F16)
            n_pre = min(4, nj)  # et tiles DMA'd during the w1t load
            ets = {}
            for t in range(n_pre):
                ets[t] = encT_pool.tile(
                    [128, KF, 128], F16, tag="et", name=f"et{t}"
                )
            hbb_sb = consts.tile([128, nb, DIM], F16)
            # batched issues: each dma_start costs ~0.6us of serial SP
            # issue time, so W1 chunks go in groups (descriptors inside one
            # issue already fan out across all 16 DMA engines); early et
            # tiles and hbb are interleaved so PE startup demand tracks
            # supply; et0's k=0 chunk goes first so the PE starts ASAP
            def w1_group(lo, hi):
                nc.sync.dma_start(out=w1t_sb[:, lo:hi, :], in_=w1t[:, lo:hi, :])

            nc.sync.dma_start(out=ets[0][:, 0:1, :], in_=encT[0][:, 0:1, :])
            w1_group(0, 2)
            nc.sync.dma_start(out=ets[0][:, 1:, :], in_=encT[0][:, 1:, :])
            if nj > 1:
                nc.sync.dma_start(out=ets[1], in_=encT[1])
            w1_group(2, 4)
            w1_group(4, 8)
            if nj > 2:
                nc.sync.dma_start(out=ets[2], in_=encT[2])
            w1_group(8, 12)
            w1_group(12, 16)
            nc.sync.dma_start(out=hbb_sb, in_=hbb[:])
            w1_group(16, 20)
            if nj > 3:
                nc.sync.dma_start(out=ets[3], in_=encT[3])
            w1_group(20, KF)
            w2b_sb = consts.tile([128, DIM], F16)
            nc.sync.dma_start(out=w2b_sb, in_=w2b[:])
            ones_sb = consts.tile([1, 128], F16)
            nc.sync.dma_start(out=ones_sb, in_=onesb[:])
            # tail-only constants declared here, loaded late (low priority)
            w3t_sb = consts.tile([128, KF, DIM], F16)
            b3_sb = consts.tile([nb, DIM], F32)

            negc_sb = consts.tile([128, 1], F32)
            nc.vector.memset(negc_sb, EXP_SHIFT)

            e_sb = consts.tile([128, nj], F32)
            e2_sb = consts.tile([128, 2], F32)
            lparts_sb = consts.tile([1, nb, j_tiles], F32)
            linv_sb = consts.tile([1, nb], F32)
            invl_sb = consts.tile([nb, 1], F32)
            ctxT_sb = consts.tile([128, KF, nb], F16)
            out_sb = consts.tile([nb, DIM], F32)

            ctx_accs = {}
            pending = None

            def emit_ctx_tail(state, split=False):
                b, j, et, wr = state
                ctx_acc = ctx_accs[b]
                # broadcast w across partitions via K=1 outer product
                wbp = ps.tile([128, 128], F32, tag="wb", bufs=2)
                nc.tensor.matmul(wbp, ones_sb, wr, start=True, stop=True)
                wb = wrow_pool.tile([128, 128], F16, tag="wb")
                nc.vector.tensor_copy(wb, wbp)
                # ctx_partial[f-chunk c] = sum_s wb[:, s] * et[:, c, s]
                # split=True emits two f-halves so the first ctxT chunks land
                # early and the W3 k-loop can start while the second half runs
                halves = [(0, KF // 2), (KF // 2, KF)] if split else [(0, KF)]
                pr = prod_pool.tile([128, KF, 128], F16)
                for lo, hi in halves:
                    nc.vector.tensor_mul(
                        pr[:, lo:hi, :], et[:, lo:hi, :], _bcast_free(wb[:], hi - lo)
                    )
                    cpart = ctxa_pool.tile(
                        [128, hi - lo], F32, tag="cpart", bufs=2, name=f"cp{lo}"
                    )
                    nc.vector.tensor_reduce(
                        out=cpart,
                        in_=pr[:, lo:hi, :],
                        axis=mybir.AxisListType.X,
                        op=mybir.AluOpType.add,
                    )
                    if j == 0:
                        nc.vector.tensor_copy(ctx_acc[:, lo:hi], cpart)
                    else:
                        nc.vector.tensor_add(
                            ctx_acc[:, lo:hi], ctx_acc[:, lo:hi], cpart
                        )
                    if j == j_tiles - 1:
                        # ctxT column for this batch (f16 for the W3 matmuls)
                        nc.vector.tensor_copy(ctxT_sb[:, lo:hi, b], ctx_acc[:, lo:hi])

            def emit_post(t, et, hps):
                """bias add + tanh + e + exp + w-row for tile t."""
                nonlocal pending
                b, j = seq[t]
                th = tanh_pool.tile([128, DIM], F16)
                for nh in range(2):
                    sl = ds(nh * 512, 512)
                    nc.vector.tensor_add(hps[nh], hps[nh], hbb_sb[:, b, sl])
                    nc.scalar.activation(
                        th[:, sl], hps[nh], mybir.ActivationFunctionType.Tanh
                    )
                sc = scratch_pool.tile([128, DIM], F16)
                nc.vector.scalar_tensor_tensor(
                    out=sc,
                    in0=th,
                    scalar=1.0,
                    in1=w2b_sb,
                    op0=mybir.AluOpType.mult,
                    op1=mybir.AluOpType.mult,
                    accum_out=e_sb[:, t : t + 1],
                )
                # w = exp(e-4) as a column, spread to a row via DMA
                wc = wrow_pool.tile([128, 1], F16, tag="wc")
                nc.scalar.activation(
                    wc,
                    e_sb[:, t : t + 1],
                    mybir.ActivationFunctionType.Exp,
                    bias=negc_sb,
                )
                wr = wrow_pool.tile([1, 128], F16)
                nc.sync.dma_start(out=wr, in_=wc)
                nc.vector.tensor_reduce(
                    out=lparts_sb[0:1, b, j : j + 1],
                    in_=wr,
                    axis=mybir.AxisListType.X,
                    op=mybir.AluOpType.add,
                )
                if pending is not None:
                    emit_ctx_tail(pending)
                pending = (b, j, et, wr)

            for b in range(nb):
                ctx_accs[b] = ctxa_pool.tile(
                    [128, KF], F32, tag="ctx_acc", bufs=nb, name=f"ctx_acc{b}"
                )

            # ---- skew group: interleave k-loops of the first tiles so each
            # W1 chunk feeds 2*n_skew matmuls as it lands ----
            skew_ps = {
                (t, nh): ps.tile(
                    [128, 512], F32, tag="h", bufs=6, name=f"skew_ps{t}_{nh}"
                )
                for t in range(n_skew)
                for nh in range(2)
            }

            def skew_mm(t, nh, k, start, stop):
                nc.tensor.matmul(
                    skew_ps[(t, nh)],
                    ets[t][:, k, :],
                    w1t_sb[:, k, ds(nh * 512, 512)],
                    start=start,
                    stop=stop,
                )

            if n_skew == 3:
                lag = KF // 3
                for k in range(lag):
                    for t in range(2):
                        for nh in range(2):
                            skew_mm(t, nh, k, k == 0, False)
                for k in range(lag, KF):
                    for t in range(3):
                        for nh in range(2):
                            skew_mm(
                                t,
                                nh,
                                k,
                                (k == lag) if t == 2 else False,
                                (k == KF - 1) if t < 2 else False,
                            )
                for t in range(2):
                    emit_post(t, ets[t], [skew_ps[(t, 0)], skew_ps[(t, 1)]])
                for k in range(lag):
                    for nh in range(2):
                        skew_mm(2, nh, k, False, k == lag - 1)
                emit_post(2, ets[2], [skew_ps[(2, 0)], skew_ps[(2, 1)]])
            else:
                for k in range(KF):
                    for t in range(n_skew):
                        for nh in range(2):
                            skew_mm(t, nh, k, k == 0, k == KF - 1)
                for t in range(n_skew):
                    emit_post(t, ets[t], [skew_ps[(t, 0)], skew_ps[(t, 1)]])

            # ---- main loop ----
            w3_loaded = set()
            for t in range(n_skew, nj):
                if t in ets:
                    et = ets[t]
                else:
                    et = encT_pool.tile([128, KF, 128], F16, tag="et")
                    nc.sync.dma_start(out=et, in_=encT[t])
                # spread the w3t prefetch across the main loop
                if t >= min(4, nj - 1):
                    span = max(nj - min(4, nj - 1), 1)
                    pos = t - min(4, nj - 1)
                    lo, hi = pos * KF // span, (pos + 1) * KF // span
                    for kk in range(lo, min(hi, KF)):
                        nc.sync.dma_start(out=w3t_sb[:, kk, :], in_=w3t[kk])
                        w3_loaded.add(kk)
                hps = []
                for nh in range(2):
                    sl = ds(nh * 512, 512)
                    hp = ps.tile([128, 512], F32, tag="h", bufs=6)
                    hps.append(hp)
                    for k in range(KF):
                        nc.tensor.matmul(
                            hp,
                            et[:, k, :],
                            w1t_sb[:, k, sl],
                            start=(k == 0),
                            stop=(k == KF - 1),
                        )
                        if (
                            t == nj - 1
                            and nh == 0
                            and k == KF // 2
                            and pending is not None
                        ):
                            # flush the second-to-last tile's chain from mid
                            # k-loop: its wb input is long ready (no PE
                            # stall) and its DVE work then clears the queue
                            # before this tile's own bias/e/exp chain
                            emit_ctx_tail(pending)
                            pending = None
                    if t == nj - 1:
                        # pipelined last tile: each half's bias/tanh/e-dot
                        # runs under the other half's matmul stream
                        b, j = seq[t]
                        if nh == 0:
                            th_last = tanh_pool.tile([128, DIM], F16)
                            sc_last = scratch_pool.tile([128, DIM], F16)
                        nc.vector.tensor_add(hp, hp, hbb_sb[:, b, sl])
                        nc.scalar.activation(
                            th_last[:, sl], hp, mybir.ActivationFunctionType.Tanh
                        )
                        nc.vector.scalar_tensor_tensor(
                            out=sc_last[:, sl],
                            in0=th_last[:, sl],
                            scalar=1.0,
                            in1=w2b_sb[:, sl],
                            op0=mybir.AluOpType.mult,
                            op1=mybir.AluOpType.mult,
                            accum_out=e2_sb[:, nh : nh + 1],
                        )
                if t == nj - 1:
                    b, j = seq[t]
                    nc.vector.tensor_add(
                        e_sb[:, t : t + 1], e2_sb[:, 0:1], e2_sb[:, 1:2]
                    )
                    wc = wrow_pool.tile([128, 1], F16, tag="wc")
                    nc.scalar.activation(
                        wc,
                        e_sb[:, t : t + 1],
                        mybir.ActivationFunctionType.Exp,
                        bias=negc_sb,
                    )
                    wr = wrow_pool.tile([1, 128], F16)
                    nc.sync.dma_start(out=wr, in_=wc)
                    nc.vector.tensor_reduce(
                        out=lparts_sb[0:1, b, j : j + 1],
                        in_=wr,
                        axis=mybir.AxisListType.X,
                        op=mybir.AluOpType.add,
                    )
                    pending = (b, j, et, wr)
                else:
                    emit_post(t, et, hps)
            for kk in range(KF):  # stragglers (nj small or uneven spread)
                if kk not in w3_loaded:
                    nc.sync.dma_start(out=w3t_sb[:, kk, :], in_=w3t[kk])
            nc.sync.dma_start(out=b3_sb, in_=b3b[:])
            if pending is not None:
                emit_ctx_tail(pending, split=True)

            # ---- 1/l per batch, spread to a partition-column ----
            nc.vector.tensor_reduce(
                out=linv_sb,
                in_=lparts_sb,
                axis=mybir.AxisListType.X,
                op=mybir.AluOpType.add,
            )
            nc.vector.reciprocal(linv_sb, linv_sb)
            nc.sync.dma_start(out=invl_sb, in_=linv_sb[0:1, :])

            # ---- out = (ctx @ W3.T) * inv_l + b3 ----
            for nh in range(2):
                sl = ds(nh * 512, 512)
                wp = ps.tile([nb, 512], F32, tag="h", bufs=6)
                for k in range(KF):
                    nc.tensor.matmul(
                        wp,
                        ctxT_sb[:, k, :],
                        w3t_sb[:, k, sl],
                        start=(k == 0),
                        stop=(k == KF - 1),
                    )
                nc.vector.scalar_tensor_tensor(
                    out=out_sb[:, sl],
                    in0=wp,
                    scalar=invl_sb,
                    in1=b3_sb[:, sl],
                    op0=mybir.AluOpType.mult,
                    op1=mybir.AluOpType.add,
                )
            nc.sync.dma_start(out=out_d[:], in_=out_sb)

    _split_multiwaits(nc)
    return nc


def make_in_maps(hidden_state, encoder_outputs, W1, b1, w2, W3, b3, nb, j_tiles):
    """Shard + lay out the full inputs for each core. Returns list of dicts."""
    f16, f32 = np.float16, np.float32
    nj = nb * j_tiles
    s_core = j_tiles * 128

    w1t = np.ascontiguousarray(
        W1.T[:F].reshape(KF, 128, DIM).transpose(1, 0, 2)
    ).astype(f16)
    w3t = np.ascontiguousarray(W3.T.reshape(KF, 128, DIM)).astype(f16)
    w2b = np.ascontiguousarray(np.broadcast_to(w2.reshape(1, DIM), (128, DIM))).astype(
        f16
    )
    onesb = np.ones((1, 128), f16)
    b3b_full = np.ascontiguousarray(
        np.broadcast_to(b3.reshape(1, DIM), (nb, DIM))
    ).astype(f32)
    # per-batch bias rows: hb = hs @ W1h.T + b1, host-computed (tiny GEMV)
    hb_full = (
        hidden_state.astype(f32) @ W1[:, F:].T.astype(f32) + b1.astype(f32)
    ).astype(f16)

    in_maps = []
    for i in range(N_CORES):
        bs = slice(i * nb, (i + 1) * nb)
        enc_c = encoder_outputs[bs, :s_core, :]  # (nb, s_core, F)
        e5 = enc_c.reshape(nb, j_tiles, 128, KF, 128)
        # j-major tile order: tile t = j*nb + b
        encT = np.ascontiguousarray(e5.transpose(1, 0, 4, 3, 2)).astype(f16)
        hbb = np.ascontiguousarray(
            np.broadcast_to(hb_full[bs][None, :, :], (128, nb, DIM))
        )
        in_maps.append(
            {
                "encT": encT.reshape(nj, 128, KF, 128),
                "w1t": w1t,
                "w3t": w3t,
                "hbb": hbb,
                "w2b": w2b,
                "b3b": b3b_full,
                "onesb": onesb,
            }
        )
    return in_maps


_CACHE = {}


def run(hidden_state, encoder_outputs, W1, b1, w2, W3, b3, nb, j_tiles, trace=False):
    key = (nb, j_tiles)
    if key not in _CACHE:
        _CACHE[key] = build_bass(nb, j_tiles)
    nc = _CACHE[key]
    in_maps = make_in_maps(
        hidden_state, encoder_outputs, W1, b1, w2, W3, b3, nb, j_tiles
    )
    res = bass_utils.run_bass_kernel_spmd(
        nc, in_maps, list(range(N_CORES)), trace=trace
    )
    out = np.concatenate([res.results[i]["out"] for i in range(N_CORES)], axis=0)
    return out.astype(np.float32), res


def kernel(hidden_state, encoder_outputs, W1, b1, w2, W3, b3):
    hidden_state = np.asarray(hidden_state, dtype=np.float32)
    encoder_outputs = np.asarray(encoder_outputs, dtype=np.float32)
    W1 = np.asarray(W1, dtype=np.float32)
    b1 = np.asarray(b1, dtype=np.float32)
    w2 = np.asarray(w2, dtype=np.float32)
    W3 = np.asarray(W3, dtype=np.float32)
    b3 = np.asarray(b3, dtype=np.float32)
    out, _ = run(hidden_state, encoder_outputs, W1, b1, w2, W3, b3, nb=4, j_tiles=8)
    return out


# revision 22
# speedup vs baseline: 1.0073x; 1.0073x over previous
"""Trainium2 Bass kernel for the aux-attention module.

reference (per batch b):
    inputs = concat([enc[b], broadcast(hs[b])], -1)          # (S, 4096)
    hidden = tanh(inputs @ W1.T + b1)                        # (S, 1024)
    e      = hidden @ w2.T                                   # (S,)
    alpha  = softmax(e)
    ctx    = alpha @ enc[b]                                  # (3072,)
    out[b] = ctx @ W3.T + b3                                 # (1024,)

Strategy: data-parallel over batch (4 batches/core x 8 cores), weights
replicated. PE matmuls in fp16 (fp32 PSUM). Softmax without max-subtraction:
w = exp(e - 4) unnormalized, 1/sum(w) folded into the final output scaling.

The hs@W1h + b1 per-batch bias rows (hb) are computed on HOST (tiny GEMV) and
shipped 128-replicated so the per-tile bias add runs on DVE (in-place into
PSUM) instead of costing PE outer-product matmuls. Tiles are processed
j-major (batch rotates fastest) so the four final ctx chains pipeline under
each other's matmul streams. The first two tiles' k-loops are interleaved so
early PE demand for W1 chunks (~6.3 MB) matches DMA supply.

Per core, per 128-row tile (single pass over enc, f-major layout from host):
  - hidden = tanh(enc_tile @ W1e.T + hb) : PE k-loop + DVE bias add + ACT
  - e column via one fused DVE multiply+accumulate against broadcast w2
  - w = exp(e-4) (ACT), column -> row via a tiny cross-partition DMA,
    broadcast across partitions (K=1 matmul outer product), then
    ctx_partial[f-chunk] = sum_s w[s]*enc[f, s] as a DVE multiply +
    per-chunk reduce on the same f-major tile already in SBUF. This chain is
    emitted one row-tile behind the matmul stream so the PE never waits.
Tail: inv_l via reduce+reciprocal, out = (ctxT @ W3.T) * inv_l + b3.
"""

import numpy as np

try:  # persistent compile cache: repeated runs skip the walrus compile
    import jax

    jax.config.update("jax_compilation_cache_dir", "/tmp/jax_neff_cache")
    jax.config.update("jax_persistent_cache_min_compile_time_secs", 1.0)
except Exception:
    pass

import concourse.bass as bass
import concourse.tile as tile
from concourse import mybir
from concourse.bass import ds
from concourse import bass_utils

# ---------------------------------------------------------------------------
# Walrus in this container caps sync waits per instruction (one; two for
# EventSemaphore). Tile's tail drain carries one wait per live semaphore and
# Tile occasionally leaks multi-wait instructions; split extras onto cheap
# carriers.
from concourse import tile as _tile_mod
from concourse import mybir as _mybir


def _patched_drain_and_barrier(self, tick_clock, wait_clock):
    nc = self.nc
    drain_inst = nc.sync.drain()
    wait_clock.add_sem_waits(
        drain_inst.ins, _tile_mod.ScopedClock({None: tick_clock.global_clock})
    )
    si = drain_inst.ins.sync_info
    waits = list(si.on_wait) if si is not None else []
    if len(waits) > 1:
        drain_inst.ins.sync_info = _mybir.SyncInfo(on_update=[], on_wait=waits[:1])
        for w in waits[1:]:
            extra = nc.sync.nop(nofuse=True, hint="drain_wait_split")
            extra.ins.sync_info = _mybir.SyncInfo(on_update=[], on_wait=[w])
    nc.all_engine_barrier()
    assert self.sems is not None
    popped = nc._tile_sem_poison_stack.pop()
    assert popped is self._sem_poison
    nc.clear_and_free_semaphores(list(self.sems.allocated().values()))
    nc.all_engine_barrier()


_tile_mod.TileContext._drain_and_barrier = _patched_drain_and_barrier


def _split_multiwaits(nc):
    for fn in nc.m.functions:
        for blk in fn.blocks:
            out, changed = [], False
            for inst in list(blk.instructions):
                si = inst.sync_info
                waits = list(si.on_wait) if si is not None else []
                cap = 2 if inst.opcode == "EventSemaphore" else 1
                if len(waits) > cap:
                    changed = True
                    for idx, w in enumerate(waits[:-cap]):
                        nop = _mybir.InstNoOp(
                            name=f"{inst.name}-wsplit{idx}", ins=[], outs=[]
                        )
                        nop.engine = inst.engine
                        nop.sync_info = _mybir.SyncInfo(on_update=[], on_wait=[w])
                        out.append(nop)
                    inst.sync_info = _mybir.SyncInfo(
                        on_update=list(si.on_update), on_wait=waits[-cap:]
                    )
                out.append(inst)
            if changed:
                blk.instructions = out


# ---------------------------------------------------------------------------

F16 = mybir.dt.float16
F32 = mybir.dt.float32

N_CORES = 8
B, S, DIM, F = 32, 1024, 1024, 3072  # F = enc feature dim; DIM = model dim
KF = F // 128  # 24 enc k-tiles
EXP_SHIFT = -4.0  # w = exp(e + EXP_SHIFT); e is O(1), shift keeps fp16 safe


def _bcast_free(ap, n, at=1):
    """Insert a step-0 (broadcast) free dim of size n at position `at`."""
    aps = list(ap.ap)
    aps.insert(at, [0, n])
    return bass.AP(tensor=ap.tensor, offset=ap.offset, ap=aps)


def _bcast_part(ap, p):
    """View a [1, n] row as a [1, p, n] replication via a step-0 middle dim
    (DMA source view; the dest AP's partition dim carries the fan-out)."""
    aps = list(ap.ap)
    assert aps[0][1] == 1
    return bass.AP(
        tensor=ap.tensor, offset=ap.offset, ap=[aps[0], [0, p]] + aps[1:]
    )


def build_bass(nb, j_tiles):
    """nb batches per core, j_tiles row-tiles of 128 per batch."""
    nj = nb * j_tiles
    nc = bass.Bass()
    encT = nc.declare_dram_parameter("encT", [nj, 128, KF, 128], F16, isOutput=False)
    w1t = nc.declare_dram_parameter("w1t", [128, KF, DIM], F16, isOutput=False)
    w3t = nc.declare_dram_parameter("w3t", [KF, 128, DIM], F16, isOutput=False)
    hbb = nc.declare_dram_parameter("hbb", [128, nb, DIM], F16, isOutput=False)
    w2b = nc.declare_dram_parameter("w2b", [128, DIM], F16, isOutput=False)
    b3b = nc.declare_dram_parameter("b3b", [nb, DIM], F32, isOutput=False)
    onesb = nc.declare_dram_parameter("onesb", [1, 128], F16, isOutput=False)
    eyeb = nc.declare_dram_parameter("eyeb", [128, 128], F16, isOutput=False)
    out_d = nc.declare_dram_parameter("out", [nb, DIM], F32, isOutput=True)

    # tile t -> (b, j), j-major: the last nb tiles are the final row-tile of
    # each batch, so their ctx chains pipeline under each other's matmuls.
    seq = [(t % nb, t // nb) for t in range(nj)]
    # first tiles run interleaved k-loops so PE demand for each W1 chunk
    # tracks DMA supply; with 3 tiles the third lags by `lag` k-steps (its
    # PSUM group accumulates k=lag..KF-1 first, then wraps to k=0..lag-1)
    n_skew = 3 if nj >= 4 else min(2, nj)

    with tile.TileContext(nc) as tc:
        with (
            tc.tile_pool(name="consts", bufs=1) as consts,
            tc.tile_pool(name="encT", bufs=5) as encT_pool,
            tc.tile_pool(name="tanh", bufs=3) as tanh_pool,
            tc.tile_pool(name="scratch", bufs=1) as scratch_pool,
            tc.tile_pool(name="prod", bufs=2) as prod_pool,
            tc.tile_pool(name="wrow", bufs=3) as wrow_pool,
            tc.tile_pool(name="ctxa", bufs=2) as ctxa_pool,
            tc.tile_pool(name="ps", bufs=4, space="PSUM") as ps,
        ):
            # ---- resident constants ----
            # DMA emission order is the schedule priority: the PE needs et
            # tiles for the skew group plus W1 chunks in k order, everything
            # else after.
            w1t_sb = consts.tile([128, KF, DIM], F16)
            n_pre = min(4, nj)  # et tiles DMA'd during the w1t load
            ets = {}
            for t in range(n_pre):
                ets[t] = encT_pool.tile(
                    [128, KF, 128], F16, tag="et", name=f"et{t}"
                )
            hbb_sb = consts.tile([128, nb, DIM], F16)
            # batched issues: each dma_start costs ~0.6us of serial SP
            # issue time, so W1 chunks go in groups (descriptors inside one
            # issue already fan out across all 16 DMA engines); early et
            # tiles and hbb are interleaved so PE startup demand tracks
            # supply; et0's k=0 chunk goes first so the PE starts ASAP
            def w1_group(lo, hi):
                nc.sync.dma_start(out=w1t_sb[:, lo:hi, :], in_=w1t[:, lo:hi, :])

            nc.sync.dma_start(out=ets[0][:, 0:1, :], in_=encT[0][:, 0:1, :])
            w1_group(0, 2)
            nc.sync.dma_start(out=ets[0][:, 1:, :], in_=encT[0][:, 1:, :])
            if nj > 1:
                nc.sync.dma_start(out=ets[1], in_=encT[1])
            w1_group(2, 4)
            w1_group(4, 8)
            if nj > 2:
                nc.sync.dma_start(out=ets[2], in_=encT[2])
            w1_group(8, 12)
            w1_group(12, 16)
            nc.sync.dma_start(out=hbb_sb, in_=hbb[:])
            w1_group(16, 20)
            if nj > 3:
                nc.sync.dma_start(out=ets[3], in_=encT[3])
            w1_group(20, KF)
            w2b_sb = consts.tile([128, DIM], F16)
            nc.sync.dma_start(out=w2b_sb, in_=w2b[:])
            ones_sb = consts.tile([1, 128], F16)
            nc.sync.dma_start(out=ones_sb, in_=onesb[:])
            eye_sb = consts.tile([128, 128], F16)
            nc.sync.dma_start(out=eye_sb, in_=eyeb[:])
            # tail-only constants declared here, loaded late (low priority)
            w3t_sb = consts.tile([128, KF, DIM], F16)
            b3_sb = consts.tile([nb, DIM], F32)

            negc_sb = consts.tile([128, 1], F32)
            nc.vector.memset(negc_sb, EXP_SHIFT)

            e_sb = consts.tile([128, nj], F32)
            e2_sb = consts.tile([128, 2], F32)
            lparts_sb = consts.tile([1, nb, j_tiles], F32)
            linv_sb = consts.tile([1, nb], F32)
            invl_sb = consts.tile([nb, 1], F32)
            ctxT_sb = consts.tile([128, KF, nb], F16)
            out_sb = consts.tile([nb, DIM], F32)

            ctx_accs = {}
            pending = None
            p30 = wb30 = pr30 = None

            def emit_wb_bcast(wr):
                # broadcast w across partitions via K=1 outer product
                wbp = ps.tile([128, 128], F32, tag="wb", bufs=2)
                nc.tensor.matmul(wbp, ones_sb, wr, start=True, stop=True)
                wb = wrow_pool.tile([128, 128], F16, tag="wb")
                nc.vector.tensor_copy(wb, wbp)
                return wb

            def emit_ctx_half(state, wb, pr, lo, hi):
                # ctx_partial[f-chunk c] = sum_s wb[:, s] * et[:, c, s]
                b, j, et, wr = state
                ctx_acc = ctx_accs[b]
                nc.vector.tensor_mul(
                    pr[:, lo:hi, :], et[:, lo:hi, :], _bcast_free(wb[:], hi - lo)
                )
                cpart = ctxa_pool.tile(
                    [128, hi - lo], F32, tag="cpart", bufs=2, name=f"cp{lo}"
                )
                nc.vector.tensor_reduce(
                    out=cpart,
                    in_=pr[:, lo:hi, :],
                    axis=mybir.AxisListType.X,
                    op=mybir.AluOpType.add,
                )
                if j == 0:
                    nc.vector.tensor_copy(ctx_acc[:, lo:hi], cpart)
                else:
                    nc.vector.tensor_add(ctx_acc[:, lo:hi], ctx_acc[:, lo:hi], cpart)
                if j == j_tiles - 1:
                    # ctxT column for this batch (f16 for the W3 matmuls)
                    nc.vector.tensor_copy(ctxT_sb[:, lo:hi, b], ctx_acc[:, lo:hi])

            def emit_ctx_tail(state, split=False):
                wb = emit_wb_bcast(state[3])
                pr = prod_pool.tile([128, KF, 128], F16, name="pr")
                halves = [(0, KF // 2), (KF // 2, KF)] if split else [(0, KF)]
                for lo, hi in halves:
                    emit_ctx_half(state, wb, pr, lo, hi)

            def emit_post(t, et, hps):
                """bias add + tanh + e + exp + w-row for tile t."""
                nonlocal pending
                b, j = seq[t]
                th = tanh_pool.tile([128, DIM], F16)
                for nh in range(2):
                    sl = ds(nh * 512, 512)
                    nc.vector.tensor_add(hps[nh], hps[nh], hbb_sb[:, b, sl])
                    nc.scalar.activation(
                        th[:, sl], hps[nh], mybir.ActivationFunctionType.Tanh
                    )
                sc = scratch_pool.tile([128, DIM], F16)
                nc.vector.scalar_tensor_tensor(
                    out=sc,
                    in0=th,
                    scalar=1.0,
                    in1=w2b_sb,
                    op0=mybir.AluOpType.mult,
                    op1=mybir.AluOpType.mult,
                    accum_out=e_sb[:, t : t + 1],
                )
                # w = exp(e-4) as a column, spread to a row via DMA
                wc = wrow_pool.tile([128, 1], F16, tag="wc")
                nc.scalar.activation(
                    wc,
                    e_sb[:, t : t + 1],
                    mybir.ActivationFunctionType.Exp,
                    bias=negc_sb,
                )
                wr = wrow_pool.tile([1, 128], F16)
                nc.sync.dma_start(out=wr, in_=wc)
                nc.vector.tensor_reduce(
                    out=lparts_sb[0:1, b, j : j + 1],
                    in_=wr,
                    axis=mybir.AxisListType.X,
                    op=mybir.AluOpType.add,
                )
                if pending is not None:
                    emit_ctx_tail(pending)
                pending = (b, j, et, wr)

            for b in range(nb):
                ctx_accs[b] = ctxa_pool.tile(
                    [128, KF], F32, tag="ctx_acc", bufs=nb, name=f"ctx_acc{b}"
                )

            # ---- skew group: interleave k-loops of the first tiles so each
            # W1 chunk feeds 2*n_skew matmuls as it lands ----
            skew_ps = {
                (t, nh): ps.tile(
                    [128, 512], F32, tag="h", bufs=6, name=f"skew_ps{t}_{nh}"
                )
                for t in range(n_skew)
                for nh in range(2)
            }

            def skew_mm(t, nh, k, start, stop):
                nc.tensor.matmul(
                    skew_ps[(t, nh)],
                    ets[t][:, k, :],
                    w1t_sb[:, k, ds(nh * 512, 512)],
                    start=start,
                    stop=stop,
                )

            if n_skew == 3:
                lag = KF // 3
                for k in range(lag):
                    for t in range(2):
                        for nh in range(2):
                            skew_mm(t, nh, k, k == 0, False)
                for k in range(lag, KF):
                    for t in range(3):
                        for nh in range(2):
                            skew_mm(
                                t,
                                nh,
                                k,
                                (k == lag) if t == 2 else False,
                                (k == KF - 1) if t < 2 else False,
                            )
                for t in range(2):
                    emit_post(t, ets[t], [skew_ps[(t, 0)], skew_ps[(t, 1)]])
                for k in range(lag):
                    for nh in range(2):
                        skew_mm(2, nh, k, False, k == lag - 1)
                emit_post(2, ets[2], [skew_ps[(2, 0)], skew_ps[(2, 1)]])
            else:
                for k in range(KF):
                    for t in range(n_skew):
                        for nh in range(2):
                            skew_mm(t, nh, k, k == 0, k == KF - 1)
                for t in range(n_skew):
                    emit_post(t, ets[t], [skew_ps[(t, 0)], skew_ps[(t, 1)]])

            # ---- main loop ----
            w3_loaded = set()
            for t in range(n_skew, nj):
                if t in ets:
                    et = ets[t]
                else:
                    et = encT_pool.tile([128, KF, 128], F16, tag="et")
                    nc.sync.dma_start(out=et, in_=encT[t])
                # spread the w3t prefetch across the main loop
                if t >= min(4, nj - 1):
                    span = max(nj - min(4, nj - 1), 1)
                    pos = t - min(4, nj - 1)
                    lo, hi = pos * KF // span, (pos + 1) * KF // span
                    for kk in range(lo, min(hi, KF)):
                        nc.sync.dma_start(out=w3t_sb[:, kk, :], in_=w3t[kk])
                        w3_loaded.add(kk)
                hps = []
                for nh in range(2):
                    sl = ds(nh * 512, 512)
                    hp = ps.tile([128, 512], F32, tag="h", bufs=6)
                    hps.append(hp)
                    for k in range(KF):
                        nc.tensor.matmul(
                            hp,
                            et[:, k, :],
                            w1t_sb[:, k, sl],
                            start=(k == 0),
                            stop=(k == KF - 1),
                        )
                        if (
                            t == nj - 1
                            and nh == 0
                            and k == KF // 2
                            and pending is not None
                        ):
                            # flush half of the second-to-last tile's chain
                            # from mid k-loop (its wb input is long ready, no
                            # PE stall); the other half is emitted after this
                            # tile's own bias/e ops so those aren't queued
                            # behind 5us of DVE reduce work
                            p30 = pending
                            pending = None
                            wb30 = emit_wb_bcast(p30[3])
                            pr30 = prod_pool.tile([128, KF, 128], F16, name="pr30")
                            emit_ctx_half(p30, wb30, pr30, 0, KF // 2)
                    if t == nj - 1:
                        # pipelined last tile: each half's bias/tanh/e-dot
                        # runs under the other half's matmul stream
                        b, j = seq[t]
                        if nh == 0:
                            th_last = tanh_pool.tile([128, DIM], F16)
                            sc_last = scratch_pool.tile([128, DIM], F16)
                        nc.vector.tensor_add(hp, hp, hbb_sb[:, b, sl])
                        nc.scalar.activation(
                            th_last[:, sl], hp, mybir.ActivationFunctionType.Tanh
                        )
                        nc.vector.scalar_tensor_tensor(
                            out=sc_last[:, sl],
                            in0=th_last[:, sl],
                            scalar=1.0,
                            in1=w2b_sb[:, sl],
                            op0=mybir.AluOpType.mult,
                            op1=mybir.AluOpType.mult,
                            accum_out=e2_sb[:, nh : nh + 1],
                        )
                if t == nj - 1:
                    b, j = seq[t]
                    nc.vector.tensor_add(
                        e_sb[:, t : t + 1], e2_sb[:, 0:1], e2_sb[:, 1:2]
                    )
                    if p30 is not None:
                        emit_ctx_half(p30, wb30, pr30, KF // 2, KF)
                    wc = wrow_pool.tile([128, 1], F16, tag="wc")
                    nc.scalar.activation(
                        wc,
                        e_sb[:, t : t + 1],
                        mybir.ActivationFunctionType.Exp,
                        bias=negc_sb,
                    )
                    # column -> row via PE transpose (skips the ~2-3us
                    # SBUF-to-SBUF DMA round trip on the critical tail)
                    wtp = ps.tile([1, 128], F16, tag="wb", bufs=2)
                    nc.tensor.matmul(
                        wtp, wc, eye_sb, start=True, stop=True, is_transpose=True
                    )
                    wr = wrow_pool.tile([1, 128], F16)
                    nc.vector.tensor_copy(wr, wtp)
                    nc.vector.tensor_reduce(
                        out=lparts_sb[0:1, b, j : j + 1],
                        in_=wr,
                        axis=mybir.AxisListType.X,
                        op=mybir.AluOpType.add,
                    )
                    pending = (b, j, et, wr)
                else:
                    emit_post(t, et, hps)
            for kk in range(KF):  # stragglers (nj small or uneven spread)
                if kk not in w3_loaded:
                    nc.sync.dma_start(out=w3t_sb[:, kk, :], in_=w3t[kk])
            nc.sync.dma_start(out=b3_sb, in_=b3b[:])
            if pending is not None:
                emit_ctx_tail(pending, split=True)

            # ---- 1/l per batch, spread to a partition-column ----
            nc.vector.tensor_reduce(
                out=linv_sb,
                in_=lparts_sb,
                axis=mybir.AxisListType.X,
                op=mybir.AluOpType.add,
            )
            nc.vector.reciprocal(linv_sb, linv_sb)
            nc.sync.dma_start(out=invl_sb, in_=linv_sb[0:1, :])

            # ---- out = (ctx @ W3.T) * inv_l + b3 ----
            for nh in range(2):
                sl = ds(nh * 512, 512)
                wp = ps.tile([nb, 512], F32, tag="h", bufs=6)
                for k in range(KF):
                    nc.tensor.matmul(
                        wp,
                        ctxT_sb[:, k, :],
                        w3t_sb[:, k, sl],
                        start=(k == 0),
                        stop=(k == KF - 1),
                    )
                nc.vector.scalar_tensor_tensor(
                    out=out_sb[:, sl],
                    in0=wp,
                    scalar=invl_sb,
                    in1=b3_sb[:, sl],
                    op0=mybir.AluOpType.mult,
                    op1=mybir.AluOpType.add,
                )
            nc.sync.dma_start(out=out_d[:], in_=out_sb)

    _split_multiwaits(nc)
    return nc


def make_in_maps(hidden_state, encoder_outputs, W1, b1, w2, W3, b3, nb, j_tiles):
    """Shard + lay out the full inputs for each core. Returns list of dicts."""
    f16, f32 = np.float16, np.float32
    nj = nb * j_tiles
    s_core = j_tiles * 128

    w1t = np.ascontiguousarray(
        W1.T[:F].reshape(KF, 128, DIM).transpose(1, 0, 2)
    ).astype(f16)
    w3t = np.ascontiguousarray(W3.T.reshape(KF, 128, DIM)).astype(f16)
    w2b = np.ascontiguousarray(np.broadcast_to(w2.reshape(1, DIM), (128, DIM))).astype(
        f16
    )
    onesb = np.ones((1, 128), f16)
    eyeb = np.eye(128, dtype=f16)
    b3b_full = np.ascontiguousarray(
        np.broadcast_to(b3.reshape(1, DIM), (nb, DIM))
    ).astype(f32)
    # per-batch bias rows: hb = hs @ W1h.T + b1, host-computed (tiny GEMV)
    hb_full = (
        hidden_state.astype(f32) @ W1[:, F:].T.astype(f32) + b1.astype(f32)
    ).astype(f16)

    in_maps = []
    for i in range(N_CORES):
        bs = slice(i * nb, (i + 1) * nb)
        enc_c = encoder_outputs[bs, :s_core, :]  # (nb, s_core, F)
        e5 = enc_c.reshape(nb, j_tiles, 128, KF, 128)
        # j-major tile order: tile t = j*nb + b
        encT = np.ascontiguousarray(e5.transpose(1, 0, 4, 3, 2)).astype(f16)
        hbb = np.ascontiguousarray(
            np.broadcast_to(hb_full[bs][None, :, :], (128, nb, DIM))
        )
        in_maps.append(
            {
                "encT": encT.reshape(nj, 128, KF, 128),
                "w1t": w1t,
                "w3t": w3t,
                "hbb": hbb,
                "w2b": w2b,
                "b3b": b3b_full,
                "onesb": onesb,
                "eyeb": eyeb,
            }
        )
    return in_maps


_CACHE = {}


def run(hidden_state, encoder_outputs, W1, b1, w2, W3, b3, nb, j_tiles, trace=False):
    key = (nb, j_tiles)
    if key not in _CACHE:
        _CACHE[key] = build_bass(nb, j_tiles)
    nc = _CACHE[key]
    in_maps = make_in_maps(
        hidden_state, encoder_outputs, W1, b1, w2, W3, b3, nb, j_tiles
    )
    res = bass_utils.run_bass_kernel_spmd(
        nc, in_maps, list(range(N_CORES)), trace=trace
    )
    out = np.concatenate([res.results[i]["out"] for i in range(N_CORES)], axis=0)
    return out.astype(np.float32), res


def kernel(hidden_state, encoder_outputs, W1, b1, w2, W3, b3):
    hidden_state = np.asarray(hidden_state, dtype=np.float32)
    encoder_outputs = np.asarray(encoder_outputs, dtype=np.float32)
    W1 = np.asarray(W1, dtype=np.float32)
    b1 = np.asarray(b1, dtype=np.float32)
    w2 = np.asarray(w2, dtype=np.float32)
    W3 = np.asarray(W3, dtype=np.float32)
    b3 = np.asarray(b3, dtype=np.float32)
    out, _ = run(hidden_state, encoder_outputs, W1, b1, w2, W3, b3, nb=4, j_tiles=8)
    return out


# revision 23
# speedup vs baseline: 1.0094x; 1.0021x over previous
"""Trainium2 Bass kernel for the aux-attention module.

reference (per batch b):
    inputs = concat([enc[b], broadcast(hs[b])], -1)          # (S, 4096)
    hidden = tanh(inputs @ W1.T + b1)                        # (S, 1024)
    e      = hidden @ w2.T                                   # (S,)
    alpha  = softmax(e)
    ctx    = alpha @ enc[b]                                  # (3072,)
    out[b] = ctx @ W3.T + b3                                 # (1024,)

Strategy: data-parallel over batch (4 batches/core x 8 cores), weights
replicated. PE matmuls in fp16 (fp32 PSUM). Softmax without max-subtraction:
w = exp(e - 4) unnormalized, 1/sum(w) folded into the final output scaling.

The hs@W1h + b1 per-batch bias rows (hb) are computed on HOST (tiny GEMV) and
shipped 128-replicated so the per-tile bias add runs on DVE (in-place into
PSUM) instead of costing PE outer-product matmuls. Tiles are processed
j-major (batch rotates fastest) so the four final ctx chains pipeline under
each other's matmul streams. The first two tiles' k-loops are interleaved so
early PE demand for W1 chunks (~6.3 MB) matches DMA supply.

Per core, per 128-row tile (single pass over enc, f-major layout from host):
  - hidden = tanh(enc_tile @ W1e.T + hb) : PE k-loop + DVE bias add + ACT
  - e column via one fused DVE multiply+accumulate against broadcast w2
  - w = exp(e-4) (ACT), column -> row via a tiny cross-partition DMA,
    broadcast across partitions (K=1 matmul outer product), then
    ctx_partial[f-chunk] = sum_s w[s]*enc[f, s] as a DVE multiply +
    per-chunk reduce on the same f-major tile already in SBUF. This chain is
    emitted one row-tile behind the matmul stream so the PE never waits.
Tail: inv_l via reduce+reciprocal, out = (ctxT @ W3.T) * inv_l + b3.
"""

import numpy as np

try:  # persistent compile cache: repeated runs skip the walrus compile
    import jax

    jax.config.update("jax_compilation_cache_dir", "/tmp/jax_neff_cache")
    jax.config.update("jax_persistent_cache_min_compile_time_secs", 1.0)
except Exception:
    pass

import concourse.bass as bass
import concourse.tile as tile
from concourse import mybir
from concourse.bass import ds
from concourse import bass_utils

# ---------------------------------------------------------------------------
# Walrus in this container caps sync waits per instruction (one; two for
# EventSemaphore). Tile's tail drain carries one wait per live semaphore and
# Tile occasionally leaks multi-wait instructions; split extras onto cheap
# carriers.
from concourse import tile as _tile_mod
from concourse import mybir as _mybir


def _patched_drain_and_barrier(self, tick_clock, wait_clock):
    nc = self.nc
    drain_inst = nc.sync.drain()
    wait_clock.add_sem_waits(
        drain_inst.ins, _tile_mod.ScopedClock({None: tick_clock.global_clock})
    )
    si = drain_inst.ins.sync_info
    waits = list(si.on_wait) if si is not None else []
    if len(waits) > 1:
        drain_inst.ins.sync_info = _mybir.SyncInfo(on_update=[], on_wait=waits[:1])
        for w in waits[1:]:
            extra = nc.sync.nop(nofuse=True, hint="drain_wait_split")
            extra.ins.sync_info = _mybir.SyncInfo(on_update=[], on_wait=[w])
    nc.all_engine_barrier()
    assert self.sems is not None
    popped = nc._tile_sem_poison_stack.pop()
    assert popped is self._sem_poison
    nc.clear_and_free_semaphores(list(self.sems.allocated().values()))
    nc.all_engine_barrier()


_tile_mod.TileContext._drain_and_barrier = _patched_drain_and_barrier


def _split_multiwaits(nc):
    for fn in nc.m.functions:
        for blk in fn.blocks:
            out, changed = [], False
            for inst in list(blk.instructions):
                si = inst.sync_info
                waits = list(si.on_wait) if si is not None else []
                cap = 2 if inst.opcode == "EventSemaphore" else 1
                if len(waits) > cap:
                    changed = True
                    for idx, w in enumerate(waits[:-cap]):
                        nop = _mybir.InstNoOp(
                            name=f"{inst.name}-wsplit{idx}", ins=[], outs=[]
                        )
                        nop.engine = inst.engine
                        nop.sync_info = _mybir.SyncInfo(on_update=[], on_wait=[w])
                        out.append(nop)
                    inst.sync_info = _mybir.SyncInfo(
                        on_update=list(si.on_update), on_wait=waits[-cap:]
                    )
                out.append(inst)
            if changed:
                blk.instructions = out


# ---------------------------------------------------------------------------

F16 = mybir.dt.float16
F32 = mybir.dt.float32

N_CORES = 8
B, S, DIM, F = 32, 1024, 1024, 3072  # F = enc feature dim; DIM = model dim
KF = F // 128  # 24 enc k-tiles
EXP_SHIFT = -4.0  # w = exp(e + EXP_SHIFT); e is O(1), shift keeps fp16 safe


def _bcast_free(ap, n, at=1):
    """Insert a step-0 (broadcast) free dim of size n at position `at`."""
    aps = list(ap.ap)
    aps.insert(at, [0, n])
    return bass.AP(tensor=ap.tensor, offset=ap.offset, ap=aps)


def _bcast_part(ap, p):
    """View a [1, n] row as a [1, p, n] replication via a step-0 middle dim
    (DMA source view; the dest AP's partition dim carries the fan-out)."""
    aps = list(ap.ap)
    assert aps[0][1] == 1
    return bass.AP(
        tensor=ap.tensor, offset=ap.offset, ap=[aps[0], [0, p]] + aps[1:]
    )


def build_bass(nb, j_tiles):
    """nb batches per core, j_tiles row-tiles of 128 per batch."""
    nj = nb * j_tiles
    nc = bass.Bass()
    encT = nc.declare_dram_parameter("encT", [nj, 128, KF, 128], F16, isOutput=False)
    w1t = nc.declare_dram_parameter("w1t", [128, KF, DIM], F16, isOutput=False)
    w3t = nc.declare_dram_parameter("w3t", [KF, 128, DIM], F16, isOutput=False)
    hbb = nc.declare_dram_parameter("hbb", [128, nb, DIM], F16, isOutput=False)
    w2b = nc.declare_dram_parameter("w2b", [128, DIM], F16, isOutput=False)
    b3b = nc.declare_dram_parameter("b3b", [nb, DIM], F32, isOutput=False)
    onesb = nc.declare_dram_parameter("onesb", [1, 128], F16, isOutput=False)
    eyeb = nc.declare_dram_parameter("eyeb", [128, 128], F16, isOutput=False)
    out_d = nc.declare_dram_parameter("out", [nb, DIM], F32, isOutput=True)

    # tile t -> (b, j), j-major: the last nb tiles are the final row-tile of
    # each batch, so their ctx chains pipeline under each other's matmuls.
    seq = [(t % nb, t // nb) for t in range(nj)]
    # first tiles run interleaved k-loops so PE demand for each W1 chunk
    # tracks DMA supply; with 3 tiles the third lags by `lag` k-steps (its
    # PSUM group accumulates k=lag..KF-1 first, then wraps to k=0..lag-1)
    n_skew = 3 if nj >= 4 else min(2, nj)

    with tile.TileContext(nc) as tc:
        with (
            tc.tile_pool(name="consts", bufs=1) as consts,
            tc.tile_pool(name="encT", bufs=5) as encT_pool,
            tc.tile_pool(name="tanh", bufs=3) as tanh_pool,
            tc.tile_pool(name="scratch", bufs=1) as scratch_pool,
            tc.tile_pool(name="prod", bufs=2) as prod_pool,
            tc.tile_pool(name="wrow", bufs=3) as wrow_pool,
            tc.tile_pool(name="ctxa", bufs=2) as ctxa_pool,
            tc.tile_pool(name="ps", bufs=4, space="PSUM") as ps,
        ):
            # ---- resident constants ----
            # DMA emission order is the schedule priority: the PE needs et
            # tiles for the skew group plus W1 chunks in k order, everything
            # else after.
            w1t_sb = consts.tile([128, KF, DIM], F16)
            n_pre = min(4, nj)  # et tiles DMA'd during the w1t load
            ets = {}
            for t in range(n_pre):
                ets[t] = encT_pool.tile(
                    [128, KF, 128], F16, tag="et", name=f"et{t}"
                )
            hbb_sb = consts.tile([128, nb, DIM], F16)
            # batched issues: each dma_start costs ~0.6us of serial SP
            # issue time, so W1 chunks go in groups (descriptors inside one
            # issue already fan out across all 16 DMA engines); early et
            # tiles and hbb are interleaved so PE startup demand tracks
            # supply; et0's k=0 chunk goes first so the PE starts ASAP
            def w1_group(lo, hi):
                nc.sync.dma_start(out=w1t_sb[:, lo:hi, :], in_=w1t[:, lo:hi, :])

            nc.sync.dma_start(out=ets[0][:, 0:1, :], in_=encT[0][:, 0:1, :])
            w1_group(0, 2)
            nc.sync.dma_start(out=ets[0][:, 1:, :], in_=encT[0][:, 1:, :])
            if nj > 1:
                nc.sync.dma_start(out=ets[1], in_=encT[1])
            w1_group(2, 4)
            w1_group(4, 8)
            if nj > 2:
                nc.sync.dma_start(out=ets[2], in_=encT[2])
            w1_group(8, 12)
            w1_group(12, 16)
            nc.sync.dma_start(out=hbb_sb, in_=hbb[:])
            w1_group(16, 20)
            if nj > 3:
                nc.sync.dma_start(out=ets[3], in_=encT[3])
            w1_group(20, KF)
            w2b_sb = consts.tile([128, DIM], F16)
            nc.sync.dma_start(out=w2b_sb, in_=w2b[:])
            ones_sb = consts.tile([1, 128], F16)
            nc.sync.dma_start(out=ones_sb, in_=onesb[:])
            eye_sb = consts.tile([128, 128], F16)
            nc.sync.dma_start(out=eye_sb, in_=eyeb[:])
            # tail-only constants declared here, loaded late (low priority)
            w3t_sb = consts.tile([128, KF, DIM], F16)
            b3_sb = consts.tile([nb, DIM], F32)

            negc_sb = consts.tile([128, 1], F32)
            nc.vector.memset(negc_sb, EXP_SHIFT)

            e_sb = consts.tile([128, nj], F32)
            e2_sb = consts.tile([128, 2], F32)
            lparts_sb = consts.tile([1, nb, j_tiles], F32)
            linv_sb = consts.tile([1, nb], F32)
            invl_sb = consts.tile([nb, 1], F32)
            ctxT_sb = consts.tile([128, KF, nb], F16)
            out_sb = consts.tile([nb, DIM], F32)

            ctx_accs = {}
            pending = None
            p30 = wb30 = pr30 = None

            def emit_wb_bcast(wr):
                # broadcast w across partitions via K=1 outer product
                wbp = ps.tile([128, 128], F32, tag="wb", bufs=2)
                nc.tensor.matmul(wbp, ones_sb, wr, start=True, stop=True)
                wb = wrow_pool.tile([128, 128], F16, tag="wb")
                nc.vector.tensor_copy(wb, wbp)
                return wb

            def emit_ctx_half(state, wb, pr, lo, hi):
                # ctx_partial[f-chunk c] = sum_s wb[:, s] * et[:, c, s]
                b, j, et, wr = state
                ctx_acc = ctx_accs[b]
                nc.vector.tensor_mul(
                    pr[:, lo:hi, :], et[:, lo:hi, :], _bcast_free(wb[:], hi - lo)
                )
                cpart = ctxa_pool.tile(
                    [128, hi - lo], F32, tag="cpart", bufs=2, name=f"cp{lo}"
                )
                nc.vector.tensor_reduce(
                    out=cpart,
                    in_=pr[:, lo:hi, :],
                    axis=mybir.AxisListType.X,
                    op=mybir.AluOpType.add,
                )
                if j == 0:
                    nc.vector.tensor_copy(ctx_acc[:, lo:hi], cpart)
                else:
                    nc.vector.tensor_add(ctx_acc[:, lo:hi], ctx_acc[:, lo:hi], cpart)
                if j == j_tiles - 1:
                    # ctxT column for this batch (f16 for the W3 matmuls)
                    nc.vector.tensor_copy(ctxT_sb[:, lo:hi, b], ctx_acc[:, lo:hi])

            def emit_ctx_tail(state, split=False):
                # split=True: emit in quarters so the W3 k-loop starts
                # consuming ctxT chunks while later quarters still reduce
                wb = emit_wb_bcast(state[3])
                pr = prod_pool.tile([128, KF, 128], F16, name="pr")
                step = KF // 4 if split else KF
                for lo in range(0, KF, step):
                    emit_ctx_half(state, wb, pr, lo, lo + step)

            def emit_post(t, et, hps):
                """bias add + tanh + e + exp + w-row for tile t."""
                nonlocal pending
                b, j = seq[t]
                th = tanh_pool.tile([128, DIM], F16)
                for nh in range(2):
                    sl = ds(nh * 512, 512)
                    nc.vector.tensor_add(hps[nh], hps[nh], hbb_sb[:, b, sl])
                    nc.scalar.activation(
                        th[:, sl], hps[nh], mybir.ActivationFunctionType.Tanh
                    )
                sc = scratch_pool.tile([128, DIM], F16)
                nc.vector.scalar_tensor_tensor(
                    out=sc,
                    in0=th,
                    scalar=1.0,
                    in1=w2b_sb,
                    op0=mybir.AluOpType.mult,
                    op1=mybir.AluOpType.mult,
                    accum_out=e_sb[:, t : t + 1],
                )
                # w = exp(e-4) as a column, spread to a row via DMA
                wc = wrow_pool.tile([128, 1], F16, tag="wc")
                nc.scalar.activation(
                    wc,
                    e_sb[:, t : t + 1],
                    mybir.ActivationFunctionType.Exp,
                    bias=negc_sb,
                )
                wr = wrow_pool.tile([1, 128], F16)
                nc.sync.dma_start(out=wr, in_=wc)
                nc.vector.tensor_reduce(
                    out=lparts_sb[0:1, b, j : j + 1],
                    in_=wr,
                    axis=mybir.AxisListType.X,
                    op=mybir.AluOpType.add,
                )
                if pending is not None:
                    emit_ctx_tail(pending)
                pending = (b, j, et, wr)

            for b in range(nb):
                ctx_accs[b] = ctxa_pool.tile(
                    [128, KF], F32, tag="ctx_acc", bufs=nb, name=f"ctx_acc{b}"
                )

            # ---- skew group: interleave k-loops of the first tiles so each
            # W1 chunk feeds 2*n_skew matmuls as it lands ----
            skew_ps = {
                (t, nh): ps.tile(
                    [128, 512], F32, tag="h", bufs=6, name=f"skew_ps{t}_{nh}"
                )
                for t in range(n_skew)
                for nh in range(2)
            }

            def skew_mm(t, nh, k, start, stop):
                nc.tensor.matmul(
                    skew_ps[(t, nh)],
                    ets[t][:, k, :],
                    w1t_sb[:, k, ds(nh * 512, 512)],
                    start=start,
                    stop=stop,
                )

            if n_skew == 3:
                lag = KF // 3
                for k in range(lag):
                    for t in range(2):
                        for nh in range(2):
                            skew_mm(t, nh, k, k == 0, False)
                for k in range(lag, KF):
                    for t in range(3):
                        for nh in range(2):
                            skew_mm(
                                t,
                                nh,
                                k,
                                (k == lag) if t == 2 else False,
                                (k == KF - 1) if t < 2 else False,
                            )
                for t in range(2):
                    emit_post(t, ets[t], [skew_ps[(t, 0)], skew_ps[(t, 1)]])
                for k in range(lag):
                    for nh in range(2):
                        skew_mm(2, nh, k, False, k == lag - 1)
                emit_post(2, ets[2], [skew_ps[(2, 0)], skew_ps[(2, 1)]])
            else:
                for k in range(KF):
                    for t in range(n_skew):
                        for nh in range(2):
                            skew_mm(t, nh, k, k == 0, k == KF - 1)
                for t in range(n_skew):
                    emit_post(t, ets[t], [skew_ps[(t, 0)], skew_ps[(t, 1)]])

            # ---- main loop ----
            w3_loaded = set()
            for t in range(n_skew, nj):
                if t in ets:
                    et = ets[t]
                else:
                    et = encT_pool.tile([128, KF, 128], F16, tag="et")
                    nc.sync.dma_start(out=et, in_=encT[t])
                # spread the w3t prefetch across the main loop
                if t >= min(4, nj - 1):
                    span = max(nj - min(4, nj - 1), 1)
                    pos = t - min(4, nj - 1)
                    lo, hi = pos * KF // span, (pos + 1) * KF // span
                    for kk in range(lo, min(hi, KF)):
                        nc.sync.dma_start(out=w3t_sb[:, kk, :], in_=w3t[kk])
                        w3_loaded.add(kk)
                hps = []
                for nh in range(2):
                    sl = ds(nh * 512, 512)
                    hp = ps.tile([128, 512], F32, tag="h", bufs=6)
                    hps.append(hp)
                    for k in range(KF):
                        nc.tensor.matmul(
                            hp,
                            et[:, k, :],
                            w1t_sb[:, k, sl],
                            start=(k == 0),
                            stop=(k == KF - 1),
                        )
                        if (
                            t == nj - 1
                            and nh == 0
                            and k == KF // 2
                            and pending is not None
                        ):
                            # flush half of the second-to-last tile's chain
                            # from mid k-loop (its wb input is long ready, no
                            # PE stall); the other half is emitted after this
                            # tile's own bias/e ops so those aren't queued
                            # behind 5us of DVE reduce work
                            p30 = pending
                            pending = None
                            wb30 = emit_wb_bcast(p30[3])
                            pr30 = prod_pool.tile([128, KF, 128], F16, name="pr30")
                            emit_ctx_half(p30, wb30, pr30, 0, KF // 2)
                    if t == nj - 1:
                        # pipelined last tile: each half's bias/tanh/e-dot
                        # runs under the other half's matmul stream
                        b, j = seq[t]
                        if nh == 0:
                            th_last = tanh_pool.tile([128, DIM], F16)
                            sc_last = scratch_pool.tile([128, DIM], F16)
                        nc.vector.tensor_add(hp, hp, hbb_sb[:, b, sl])
                        nc.scalar.activation(
                            th_last[:, sl], hp, mybir.ActivationFunctionType.Tanh
                        )
                        nc.vector.scalar_tensor_tensor(
                            out=sc_last[:, sl],
                            in0=th_last[:, sl],
                            scalar=1.0,
                            in1=w2b_sb[:, sl],
                            op0=mybir.AluOpType.mult,
                            op1=mybir.AluOpType.mult,
                            accum_out=e2_sb[:, nh : nh + 1],
                        )
                if t == nj - 1:
                    b, j = seq[t]
                    nc.vector.tensor_add(
                        e_sb[:, t : t + 1], e2_sb[:, 0:1], e2_sb[:, 1:2]
                    )
                    if p30 is not None:
                        emit_ctx_half(p30, wb30, pr30, KF // 2, KF)
                    wc = wrow_pool.tile([128, 1], F16, tag="wc")
                    nc.scalar.activation(
                        wc,
                        e_sb[:, t : t + 1],
                        mybir.ActivationFunctionType.Exp,
                        bias=negc_sb,
                    )
                    # column -> row via PE transpose (skips the ~2-3us
                    # SBUF-to-SBUF DMA round trip on the critical tail)
                    wtp = ps.tile([1, 128], F16, tag="wb", bufs=2)
                    nc.tensor.matmul(
                        wtp, wc, eye_sb, start=True, stop=True, is_transpose=True
                    )
                    wr = wrow_pool.tile([1, 128], F16)
                    nc.vector.tensor_copy(wr, wtp)
                    nc.vector.tensor_reduce(
                        out=lparts_sb[0:1, b, j : j + 1],
                        in_=wr,
                        axis=mybir.AxisListType.X,
                        op=mybir.AluOpType.add,
                    )
                    pending = (b, j, et, wr)
                else:
                    emit_post(t, et, hps)
            for kk in range(KF):  # stragglers (nj small or uneven spread)
                if kk not in w3_loaded:
                    nc.sync.dma_start(out=w3t_sb[:, kk, :], in_=w3t[kk])
            nc.sync.dma_start(out=b3_sb, in_=b3b[:])
            if pending is not None:
                emit_ctx_tail(pending, split=True)

            # ---- 1/l per batch, spread to a partition-column ----
            nc.vector.tensor_reduce(
                out=linv_sb,
                in_=lparts_sb,
                axis=mybir.AxisListType.X,
                op=mybir.AluOpType.add,
            )
            nc.vector.reciprocal(linv_sb, linv_sb)
            nc.sync.dma_start(out=invl_sb, in_=linv_sb[0:1, :])

            # ---- out = (ctx @ W3.T) * inv_l + b3 ----
            for nh in range(2):
                sl = ds(nh * 512, 512)
                wp = ps.tile([nb, 512], F32, tag="h", bufs=6)
                for k in range(KF):
                    nc.tensor.matmul(
                        wp,
                        ctxT_sb[:, k, :],
                        w3t_sb[:, k, sl],
                        start=(k == 0),
                        stop=(k == KF - 1),
                    )
                nc.vector.scalar_tensor_tensor(
                    out=out_sb[:, sl],
                    in0=wp,
                    scalar=invl_sb,
                    in1=b3_sb[:, sl],
                    op0=mybir.AluOpType.mult,
                    op1=mybir.AluOpType.add,
                )
            nc.sync.dma_start(out=out_d[:], in_=out_sb)

    _split_multiwaits(nc)
    return nc


def make_in_maps(hidden_state, encoder_outputs, W1, b1, w2, W3, b3, nb, j_tiles):
    """Shard + lay out the full inputs for each core. Returns list of dicts."""
    f16, f32 = np.float16, np.float32
    nj = nb * j_tiles
    s_core = j_tiles * 128

    w1t = np.ascontiguousarray(
        W1.T[:F].reshape(KF, 128, DIM).transpose(1, 0, 2)
    ).astype(f16)
    w3t = np.ascontiguousarray(W3.T.reshape(KF, 128, DIM)).astype(f16)
    w2b = np.ascontiguousarray(np.broadcast_to(w2.reshape(1, DIM), (128, DIM))).astype(
        f16
    )
    onesb = np.ones((1, 128), f16)
    eyeb = np.eye(128, dtype=f16)
    b3b_full = np.ascontiguousarray(
        np.broadcast_to(b3.reshape(1, DIM), (nb, DIM))
    ).astype(f32)
    # per-batch bias rows: hb = hs @ W1h.T + b1, host-computed (tiny GEMV)
    hb_full = (
        hidden_state.astype(f32) @ W1[:, F:].T.astype(f32) + b1.astype(f32)
    ).astype(f16)

    in_maps = []
    for i in range(N_CORES):
        bs = slice(i * nb, (i + 1) * nb)
        enc_c = encoder_outputs[bs, :s_core, :]  # (nb, s_core, F)
        e5 = enc_c.reshape(nb, j_tiles, 128, KF, 128)
        # j-major tile order: tile t = j*nb + b
        encT = np.ascontiguousarray(e5.transpose(1, 0, 4, 3, 2)).astype(f16)
        hbb = np.ascontiguousarray(
            np.broadcast_to(hb_full[bs][None, :, :], (128, nb, DIM))
        )
        in_maps.append(
            {
                "encT": encT.reshape(nj, 128, KF, 128),
                "w1t": w1t,
                "w3t": w3t,
                "hbb": hbb,
                "w2b": w2b,
                "b3b": b3b_full,
                "onesb": onesb,
                "eyeb": eyeb,
            }
        )
    return in_maps


_CACHE = {}


def run(hidden_state, encoder_outputs, W1, b1, w2, W3, b3, nb, j_tiles, trace=False):
    key = (nb, j_tiles)
    if key not in _CACHE:
        _CACHE[key] = build_bass(nb, j_tiles)
    nc = _CACHE[key]
    in_maps = make_in_maps(
        hidden_state, encoder_outputs, W1, b1, w2, W3, b3, nb, j_tiles
    )
    res = bass_utils.run_bass_kernel_spmd(
        nc, in_maps, list(range(N_CORES)), trace=trace
    )
    out = np.concatenate([res.results[i]["out"] for i in range(N_CORES)], axis=0)
    return out.astype(np.float32), res


def kernel(hidden_state, encoder_outputs, W1, b1, w2, W3, b3):
    hidden_state = np.asarray(hidden_state, dtype=np.float32)
    encoder_outputs = np.asarray(encoder_outputs, dtype=np.float32)
    W1 = np.asarray(W1, dtype=np.float32)
    b1 = np.asarray(b1, dtype=np.float32)
    w2 = np.asarray(w2, dtype=np.float32)
    W3 = np.asarray(W3, dtype=np.float32)
    b3 = np.asarray(b3, dtype=np.float32)
    out, _ = run(hidden_state, encoder_outputs, W1, b1, w2, W3, b3, nb=4, j_tiles=8)
    return out


# revision 24
# speedup vs baseline: 1.0231x; 1.0135x over previous
"""Trainium2 Bass kernel for the aux-attention module.

reference (per batch b):
    inputs = concat([enc[b], broadcast(hs[b])], -1)          # (S, 4096)
    hidden = tanh(inputs @ W1.T + b1)                        # (S, 1024)
    e      = hidden @ w2.T                                   # (S,)
    alpha  = softmax(e)
    ctx    = alpha @ enc[b]                                  # (3072,)
    out[b] = ctx @ W3.T + b3                                 # (1024,)

Strategy: data-parallel over batch (4 batches/core x 8 cores), weights
replicated. PE matmuls in fp16 (fp32 PSUM). Softmax without max-subtraction:
w = exp(e - 4) unnormalized, 1/sum(w) folded into the final output scaling.

The hs@W1h + b1 per-batch bias rows (hb) are computed on HOST (tiny GEMV) and
shipped 128-replicated so the per-tile bias add runs on DVE (in-place into
PSUM) instead of costing PE outer-product matmuls. Tiles are processed
j-major (batch rotates fastest) so the four final ctx chains pipeline under
each other's matmul streams. The first two tiles' k-loops are interleaved so
early PE demand for W1 chunks (~6.3 MB) matches DMA supply.

Per core, per 128-row tile (single pass over enc, f-major layout from host):
  - hidden = tanh(enc_tile @ W1e.T + hb) : PE k-loop + DVE bias add + ACT
  - e column via one fused DVE multiply+accumulate against broadcast w2
  - w = exp(e-4) (ACT), column -> row via a tiny cross-partition DMA,
    broadcast across partitions (K=1 matmul outer product), then
    ctx_partial[f-chunk] = sum_s w[s]*enc[f, s] as a DVE multiply +
    per-chunk reduce on the same f-major tile already in SBUF. This chain is
    emitted one row-tile behind the matmul stream so the PE never waits.
Tail: inv_l via reduce+reciprocal, out = (ctxT @ W3.T) * inv_l + b3.
"""

import numpy as np

try:  # persistent compile cache: repeated runs skip the walrus compile
    import jax

    jax.config.update("jax_compilation_cache_dir", "/tmp/jax_neff_cache")
    jax.config.update("jax_persistent_cache_min_compile_time_secs", 1.0)
except Exception:
    pass

import concourse.bass as bass
import concourse.tile as tile
from concourse import mybir
from concourse.bass import ds
from concourse import bass_utils

# ---------------------------------------------------------------------------
# Walrus in this container caps sync waits per instruction (one; two for
# EventSemaphore). Tile's tail drain carries one wait per live semaphore and
# Tile occasionally leaks multi-wait instructions; split extras onto cheap
# carriers.
from concourse import tile as _tile_mod
from concourse import mybir as _mybir


def _patched_drain_and_barrier(self, tick_clock, wait_clock):
    nc = self.nc
    drain_inst = nc.sync.drain()
    wait_clock.add_sem_waits(
        drain_inst.ins, _tile_mod.ScopedClock({None: tick_clock.global_clock})
    )
    si = drain_inst.ins.sync_info
    waits = list(si.on_wait) if si is not None else []
    if len(waits) > 1:
        drain_inst.ins.sync_info = _mybir.SyncInfo(on_update=[], on_wait=waits[:1])
        for w in waits[1:]:
            extra = nc.sync.nop(nofuse=True, hint="drain_wait_split")
            extra.ins.sync_info = _mybir.SyncInfo(on_update=[], on_wait=[w])
    nc.all_engine_barrier()
    assert self.sems is not None
    popped = nc._tile_sem_poison_stack.pop()
    assert popped is self._sem_poison
    nc.clear_and_free_semaphores(list(self.sems.allocated().values()))
    nc.all_engine_barrier()


_tile_mod.TileContext._drain_and_barrier = _patched_drain_and_barrier


def _split_multiwaits(nc):
    for fn in nc.m.functions:
        for blk in fn.blocks:
            out, changed = [], False
            for inst in list(blk.instructions):
                si = inst.sync_info
                waits = list(si.on_wait) if si is not None else []
                cap = 2 if inst.opcode == "EventSemaphore" else 1
                if len(waits) > cap:
                    changed = True
                    for idx, w in enumerate(waits[:-cap]):
                        nop = _mybir.InstNoOp(
                            name=f"{inst.name}-wsplit{idx}", ins=[], outs=[]
                        )
                        nop.engine = inst.engine
                        nop.sync_info = _mybir.SyncInfo(on_update=[], on_wait=[w])
                        out.append(nop)
                    inst.sync_info = _mybir.SyncInfo(
                        on_update=list(si.on_update), on_wait=waits[-cap:]
                    )
                out.append(inst)
            if changed:
                blk.instructions = out


# ---------------------------------------------------------------------------

F16 = mybir.dt.float16
F32 = mybir.dt.float32

N_CORES = 8
B, S, DIM, F = 32, 1024, 1024, 3072  # F = enc feature dim; DIM = model dim
KF = F // 128  # 24 enc k-tiles
EXP_SHIFT = -4.0  # w = exp(e + EXP_SHIFT); e is O(1), shift keeps fp16 safe


def _bcast_free(ap, n, at=1):
    """Insert a step-0 (broadcast) free dim of size n at position `at`."""
    aps = list(ap.ap)
    aps.insert(at, [0, n])
    return bass.AP(tensor=ap.tensor, offset=ap.offset, ap=aps)


def _bcast_part(ap, p):
    """View a [1, n] row as a [1, p, n] replication via a step-0 middle dim
    (DMA source view; the dest AP's partition dim carries the fan-out)."""
    aps = list(ap.ap)
    assert aps[0][1] == 1
    return bass.AP(
        tensor=ap.tensor, offset=ap.offset, ap=[aps[0], [0, p]] + aps[1:]
    )


def build_bass(nb, j_tiles):
    """nb batches per core, j_tiles row-tiles of 128 per batch."""
    nj = nb * j_tiles
    nc = bass.Bass()
    encT = nc.declare_dram_parameter("encT", [nj, 128, KF, 128], F16, isOutput=False)
    w1t = nc.declare_dram_parameter("w1t", [128, KF, DIM], F16, isOutput=False)
    w3t = nc.declare_dram_parameter("w3t", [128, KF, DIM], F16, isOutput=False)
    hbb = nc.declare_dram_parameter("hbb", [128, nb, DIM], F16, isOutput=False)
    w2b = nc.declare_dram_parameter("w2b", [128, DIM], F16, isOutput=False)
    b3b = nc.declare_dram_parameter("b3b", [nb, DIM], F32, isOutput=False)
    onesb = nc.declare_dram_parameter("onesb", [1, 128], F16, isOutput=False)
    eyeb = nc.declare_dram_parameter("eyeb", [128, 128], F16, isOutput=False)
    out_d = nc.declare_dram_parameter("out", [nb, DIM], F32, isOutput=True)

    # tile t -> (b, j), j-major: the last nb tiles are the final row-tile of
    # each batch, so their ctx chains pipeline under each other's matmuls.
    seq = [(t % nb, t // nb) for t in range(nj)]
    # first tiles run interleaved k-loops so PE demand for each W1 chunk
    # tracks DMA supply; with 3 tiles the third lags by `lag` k-steps (its
    # PSUM group accumulates k=lag..KF-1 first, then wraps to k=0..lag-1)
    n_skew = 3 if nj >= 4 else min(2, nj)

    with tile.TileContext(nc) as tc:
        with (
            tc.tile_pool(name="consts", bufs=1) as consts,
            tc.tile_pool(name="encT", bufs=5) as encT_pool,
            tc.tile_pool(name="tanh", bufs=3) as tanh_pool,
            tc.tile_pool(name="scratch", bufs=1) as scratch_pool,
            tc.tile_pool(name="prod", bufs=2) as prod_pool,
            tc.tile_pool(name="wrow", bufs=3) as wrow_pool,
            tc.tile_pool(name="ctxa", bufs=2) as ctxa_pool,
            tc.tile_pool(name="ps", bufs=4, space="PSUM") as ps,
        ):
            # ---- resident constants ----
            # DMA emission order is the schedule priority: the PE needs et
            # tiles for the skew group plus W1 chunks in k order, everything
            # else after.
            w1t_sb = consts.tile([128, KF, DIM], F16)
            n_pre = min(4, nj)  # et tiles DMA'd during the w1t load
            ets = {}
            for t in range(n_pre):
                ets[t] = encT_pool.tile(
                    [128, KF, 128], F16, tag="et", name=f"et{t}"
                )
            hbb_sb = consts.tile([128, nb, DIM], F16)
            # batched issues: each dma_start costs ~0.6us of serial SP
            # issue time, so W1 chunks go in groups (descriptors inside one
            # issue already fan out across all 16 DMA engines); early et
            # tiles and hbb are interleaved so PE startup demand tracks
            # supply; et0's k=0 chunk goes first so the PE starts ASAP
            def w1_group(lo, hi):
                nc.sync.dma_start(out=w1t_sb[:, lo:hi, :], in_=w1t[:, lo:hi, :])

            nc.sync.dma_start(out=ets[0][:, 0:1, :], in_=encT[0][:, 0:1, :])
            w1_group(0, 2)
            nc.sync.dma_start(out=ets[0][:, 1:, :], in_=encT[0][:, 1:, :])
            if nj > 1:
                nc.sync.dma_start(out=ets[1], in_=encT[1])
            w1_group(2, 4)
            w1_group(4, 8)
            if nj > 2:
                nc.sync.dma_start(out=ets[2], in_=encT[2])
            w1_group(8, 12)
            w1_group(12, 16)
            nc.sync.dma_start(out=hbb_sb, in_=hbb[:])
            w1_group(16, 20)
            if nj > 3:
                nc.sync.dma_start(out=ets[3], in_=encT[3])
            w1_group(20, KF)
            w2b_sb = consts.tile([128, DIM], F16)
            nc.sync.dma_start(out=w2b_sb, in_=w2b[:])
            ones_sb = consts.tile([1, 128], F16)
            nc.sync.dma_start(out=ones_sb, in_=onesb[:])
            eye_sb = consts.tile([128, 128], F16)
            nc.sync.dma_start(out=eye_sb, in_=eyeb[:])
            # tail-only constants declared here, loaded late (low priority)
            w3t_sb = consts.tile([128, KF, DIM], F16)
            b3_sb = consts.tile([nb, DIM], F32)

            negc_sb = consts.tile([128, 1], F32)
            nc.vector.memset(negc_sb, EXP_SHIFT)

            e_sb = consts.tile([128, nj], F32)
            e2_sb = consts.tile([128, 2], F32)
            lparts_sb = consts.tile([1, nb, j_tiles], F32)
            linv_sb = consts.tile([1, nb], F32)
            invl_sb = consts.tile([nb, 1], F32)
            ctxT_sb = consts.tile([128, KF, nb], F16)
            out_sb = consts.tile([nb, DIM], F32)

            ctx_accs = {}
            pending = None
            p30 = wb30 = pr30 = None

            def emit_wb_bcast(wr):
                # broadcast w across partitions via K=1 outer product
                wbp = ps.tile([128, 128], F32, tag="wb", bufs=2)
                nc.tensor.matmul(wbp, ones_sb, wr, start=True, stop=True)
                wb = wrow_pool.tile([128, 128], F16, tag="wb")
                nc.vector.tensor_copy(wb, wbp)
                return wb

            def emit_ctx_half(state, wb, pr, lo, hi):
                # ctx_partial[f-chunk c] = sum_s wb[:, s] * et[:, c, s]
                b, j, et, wr = state
                ctx_acc = ctx_accs[b]
                nc.vector.tensor_mul(
                    pr[:, lo:hi, :], et[:, lo:hi, :], _bcast_free(wb[:], hi - lo)
                )
                cpart = ctxa_pool.tile(
                    [128, hi - lo], F32, tag="cpart", bufs=2, name=f"cp{lo}"
                )
                nc.vector.tensor_reduce(
                    out=cpart,
                    in_=pr[:, lo:hi, :],
                    axis=mybir.AxisListType.X,
                    op=mybir.AluOpType.add,
                )
                if j == 0:
                    nc.vector.tensor_copy(ctx_acc[:, lo:hi], cpart)
                else:
                    nc.vector.tensor_add(ctx_acc[:, lo:hi], ctx_acc[:, lo:hi], cpart)
                if j == j_tiles - 1:
                    # ctxT column for this batch (f16 for the W3 matmuls)
                    nc.vector.tensor_copy(ctxT_sb[:, lo:hi, b], ctx_acc[:, lo:hi])

            def emit_ctx_tail(state, split=False):
                # split=True: emit in quarters so the W3 k-loop starts
                # consuming ctxT chunks while later quarters still reduce
                wb = emit_wb_bcast(state[3])
                pr = prod_pool.tile([128, KF, 128], F16, name="pr")
                step = KF // 4 if split else KF
                for lo in range(0, KF, step):
                    emit_ctx_half(state, wb, pr, lo, lo + step)

            def emit_post(t, et, hps):
                """bias add + tanh + e + exp + w-row for tile t."""
                nonlocal pending
                b, j = seq[t]
                th = tanh_pool.tile([128, DIM], F16)
                for nh in range(2):
                    sl = ds(nh * 512, 512)
                    nc.vector.tensor_add(hps[nh], hps[nh], hbb_sb[:, b, sl])
                    nc.scalar.activation(
                        th[:, sl], hps[nh], mybir.ActivationFunctionType.Tanh
                    )
                sc = scratch_pool.tile([128, DIM], F16)
                nc.vector.scalar_tensor_tensor(
                    out=sc,
                    in0=th,
                    scalar=1.0,
                    in1=w2b_sb,
                    op0=mybir.AluOpType.mult,
                    op1=mybir.AluOpType.mult,
                    accum_out=e_sb[:, t : t + 1],
                )
                # w = exp(e-4) as a column, spread to a row via DMA
                wc = wrow_pool.tile([128, 1], F16, tag="wc")
                nc.scalar.activation(
                    wc,
                    e_sb[:, t : t + 1],
                    mybir.ActivationFunctionType.Exp,
                    bias=negc_sb,
                )
                wr = wrow_pool.tile([1, 128], F16)
                nc.sync.dma_start(out=wr, in_=wc)
                nc.vector.tensor_reduce(
                    out=lparts_sb[0:1, b, j : j + 1],
                    in_=wr,
                    axis=mybir.AxisListType.X,
                    op=mybir.AluOpType.add,
                )
                if pending is not None:
                    emit_ctx_tail(pending)
                pending = (b, j, et, wr)

            for b in range(nb):
                ctx_accs[b] = ctxa_pool.tile(
                    [128, KF], F32, tag="ctx_acc", bufs=nb, name=f"ctx_acc{b}"
                )

            # ---- skew group: interleave k-loops of the first tiles so each
            # W1 chunk feeds 2*n_skew matmuls as it lands ----
            skew_ps = {
                (t, nh): ps.tile(
                    [128, 512], F32, tag="h", bufs=6, name=f"skew_ps{t}_{nh}"
                )
                for t in range(n_skew)
                for nh in range(2)
            }

            def skew_mm(t, nh, k, start, stop):
                nc.tensor.matmul(
                    skew_ps[(t, nh)],
                    ets[t][:, k, :],
                    w1t_sb[:, k, ds(nh * 512, 512)],
                    start=start,
                    stop=stop,
                )

            if n_skew == 3:
                lag = KF // 3
                for k in range(lag):
                    for t in range(2):
                        for nh in range(2):
                            skew_mm(t, nh, k, k == 0, False)
                for k in range(lag, KF):
                    for t in range(3):
                        for nh in range(2):
                            skew_mm(
                                t,
                                nh,
                                k,
                                (k == lag) if t == 2 else False,
                                (k == KF - 1) if t < 2 else False,
                            )
                for t in range(2):
                    emit_post(t, ets[t], [skew_ps[(t, 0)], skew_ps[(t, 1)]])
                for k in range(lag):
                    for nh in range(2):
                        skew_mm(2, nh, k, False, k == lag - 1)
                emit_post(2, ets[2], [skew_ps[(2, 0)], skew_ps[(2, 1)]])
            else:
                for k in range(KF):
                    for t in range(n_skew):
                        for nh in range(2):
                            skew_mm(t, nh, k, k == 0, k == KF - 1)
                for t in range(n_skew):
                    emit_post(t, ets[t], [skew_ps[(t, 0)], skew_ps[(t, 1)]])

            # ---- main loop ----
            w3_loaded = set()
            for t in range(n_skew, nj):
                if t in ets:
                    et = ets[t]
                else:
                    et = encT_pool.tile([128, KF, 128], F16, tag="et")
                    nc.sync.dma_start(out=et, in_=encT[t])
                # spread the w3t prefetch across the main loop in 4-chunk
                # groups (one SP issue each)
                if t >= min(4, nj - 1):
                    span = max(nj - min(4, nj - 1), 1)
                    pos = t - min(4, nj - 1)
                    lo, hi = pos * KF // span, (pos + 1) * KF // span
                    lo, hi = (lo + 3) // 4 * 4, (hi + 3) // 4 * 4
                    for kk in range(lo, min(hi, KF), 4):
                        nc.sync.dma_start(
                            out=w3t_sb[:, kk : kk + 4, :],
                            in_=w3t[:, kk : kk + 4, :],
                        )
                        w3_loaded.update(range(kk, kk + 4))
                hps = []
                for nh in range(2):
                    sl = ds(nh * 512, 512)
                    hp = ps.tile([128, 512], F32, tag="h", bufs=6)
                    hps.append(hp)
                    for k in range(KF):
                        nc.tensor.matmul(
                            hp,
                            et[:, k, :],
                            w1t_sb[:, k, sl],
                            start=(k == 0),
                            stop=(k == KF - 1),
                        )
                        if (
                            t == nj - 1
                            and nh == 0
                            and k == KF // 2
                            and pending is not None
                        ):
                            # flush half of the second-to-last tile's chain
                            # from mid k-loop (its wb input is long ready, no
                            # PE stall); the other half is emitted after this
                            # tile's own bias/e ops so those aren't queued
                            # behind 5us of DVE reduce work
                            p30 = pending
                            pending = None
                            wb30 = emit_wb_bcast(p30[3])
                            pr30 = prod_pool.tile([128, KF, 128], F16, name="pr30")
                            emit_ctx_half(p30, wb30, pr30, 0, KF // 2)
                    if t == nj - 1:
                        # pipelined last tile: each half's bias/tanh/e-dot
                        # runs under the other half's matmul stream
                        b, j = seq[t]
                        if nh == 0:
                            th_last = tanh_pool.tile([128, DIM], F16)
                            sc_last = scratch_pool.tile([128, DIM], F16)
                        nc.vector.tensor_add(hp, hp, hbb_sb[:, b, sl])
                        nc.scalar.activation(
                            th_last[:, sl], hp, mybir.ActivationFunctionType.Tanh
                        )
                        nc.vector.scalar_tensor_tensor(
                            out=sc_last[:, sl],
                            in0=th_last[:, sl],
                            scalar=1.0,
                            in1=w2b_sb[:, sl],
                            op0=mybir.AluOpType.mult,
                            op1=mybir.AluOpType.mult,
                            accum_out=e2_sb[:, nh : nh + 1],
                        )
                if t == nj - 1:
                    b, j = seq[t]
                    nc.vector.tensor_add(
                        e_sb[:, t : t + 1], e2_sb[:, 0:1], e2_sb[:, 1:2]
                    )
                    if p30 is not None:
                        emit_ctx_half(p30, wb30, pr30, KF // 2, KF)
                    # keep the PE clock hot through the serial exp/ctx
                    # window: redundant matmuls into a scratch psum (their
                    # inputs are resident, so they run during the idle gap)
                    for wk in range(12):
                        wp_warm = ps.tile(
                            [128, 512], F32, tag="h", bufs=6, name=f"warm{wk}"
                        )
                        nc.tensor.matmul(
                            wp_warm,
                            et[:, wk, :],
                            w1t_sb[:, wk, ds(0, 512)],
                            start=True,
                            stop=True,
                        )
                    wc = wrow_pool.tile([128, 1], F16, tag="wc")
                    nc.scalar.activation(
                        wc,
                        e_sb[:, t : t + 1],
                        mybir.ActivationFunctionType.Exp,
                        bias=negc_sb,
                    )
                    # column -> row via PE transpose (skips the ~2-3us
                    # SBUF-to-SBUF DMA round trip on the critical tail)
                    wtp = ps.tile([1, 128], F16, tag="wb", bufs=2)
                    nc.tensor.matmul(
                        wtp, wc, eye_sb, start=True, stop=True, is_transpose=True
                    )
                    wr = wrow_pool.tile([1, 128], F16)
                    nc.vector.tensor_copy(wr, wtp)
                    nc.vector.tensor_reduce(
                        out=lparts_sb[0:1, b, j : j + 1],
                        in_=wr,
                        axis=mybir.AxisListType.X,
                        op=mybir.AluOpType.add,
                    )
                    pending = (b, j, et, wr)
                else:
                    emit_post(t, et, hps)
            for kk in range(KF):  # stragglers (nj small or uneven spread)
                if kk not in w3_loaded:
                    nc.sync.dma_start(out=w3t_sb[:, kk, :], in_=w3t[:, kk, :])
            nc.sync.dma_start(out=b3_sb, in_=b3b[:])
            if pending is not None:
                emit_ctx_tail(pending, split=True)

            # ---- 1/l per batch, spread to a partition-column ----
            nc.vector.tensor_reduce(
                out=linv_sb,
                in_=lparts_sb,
                axis=mybir.AxisListType.X,
                op=mybir.AluOpType.add,
            )
            nc.vector.reciprocal(linv_sb, linv_sb)
            nc.sync.dma_start(out=invl_sb, in_=linv_sb[0:1, :])

            # ---- out = (ctx @ W3.T) * inv_l + b3 ----
            for nh in range(2):
                sl = ds(nh * 512, 512)
                wp = ps.tile([nb, 512], F32, tag="h", bufs=6)
                for k in range(KF):
                    nc.tensor.matmul(
                        wp,
                        ctxT_sb[:, k, :],
                        w3t_sb[:, k, sl],
                        start=(k == 0),
                        stop=(k == KF - 1),
                    )
                nc.vector.scalar_tensor_tensor(
                    out=out_sb[:, sl],
                    in0=wp,
                    scalar=invl_sb,
                    in1=b3_sb[:, sl],
                    op0=mybir.AluOpType.mult,
                    op1=mybir.AluOpType.add,
                )
            nc.sync.dma_start(out=out_d[:], in_=out_sb)

    _split_multiwaits(nc)
    return nc


def make_in_maps(hidden_state, encoder_outputs, W1, b1, w2, W3, b3, nb, j_tiles):
    """Shard + lay out the full inputs for each core. Returns list of dicts."""
    f16, f32 = np.float16, np.float32
    nj = nb * j_tiles
    s_core = j_tiles * 128

    w1t = np.ascontiguousarray(
        W1.T[:F].reshape(KF, 128, DIM).transpose(1, 0, 2)
    ).astype(f16)
    w3t = np.ascontiguousarray(
        W3.T.reshape(KF, 128, DIM).transpose(1, 0, 2)
    ).astype(f16)
    w2b = np.ascontiguousarray(np.broadcast_to(w2.reshape(1, DIM), (128, DIM))).astype(
        f16
    )
    onesb = np.ones((1, 128), f16)
    eyeb = np.eye(128, dtype=f16)
    b3b_full = np.ascontiguousarray(
        np.broadcast_to(b3.reshape(1, DIM), (nb, DIM))
    ).astype(f32)
    # per-batch bias rows: hb = hs @ W1h.T + b1, host-computed (tiny GEMV)
    hb_full = (
        hidden_state.astype(f32) @ W1[:, F:].T.astype(f32) + b1.astype(f32)
    ).astype(f16)

    in_maps = []
    for i in range(N_CORES):
        bs = slice(i * nb, (i + 1) * nb)
        enc_c = encoder_outputs[bs, :s_core, :]  # (nb, s_core, F)
        e5 = enc_c.reshape(nb, j_tiles, 128, KF, 128)
        # j-major tile order: tile t = j*nb + b
        encT = np.ascontiguousarray(e5.transpose(1, 0, 4, 3, 2)).astype(f16)
        hbb = np.ascontiguousarray(
            np.broadcast_to(hb_full[bs][None, :, :], (128, nb, DIM))
        )
        in_maps.append(
            {
                "encT": encT.reshape(nj, 128, KF, 128),
                "w1t": w1t,
                "w3t": w3t,
                "hbb": hbb,
                "w2b": w2b,
                "b3b": b3b_full,
                "onesb": onesb,
                "eyeb": eyeb,
            }
        )
    return in_maps


_CACHE = {}


def run(hidden_state, encoder_outputs, W1, b1, w2, W3, b3, nb, j_tiles, trace=False):
    key = (nb, j_tiles)
    if key not in _CACHE:
        _CACHE[key] = build_bass(nb, j_tiles)
    nc = _CACHE[key]
    in_maps = make_in_maps(
        hidden_state, encoder_outputs, W1, b1, w2, W3, b3, nb, j_tiles
    )
    res = bass_utils.run_bass_kernel_spmd(
        nc, in_maps, list(range(N_CORES)), trace=trace
    )
    out = np.concatenate([res.results[i]["out"] for i in range(N_CORES)], axis=0)
    return out.astype(np.float32), res


def kernel(hidden_state, encoder_outputs, W1, b1, w2, W3, b3):
    hidden_state = np.asarray(hidden_state, dtype=np.float32)
    encoder_outputs = np.asarray(encoder_outputs, dtype=np.float32)
    W1 = np.asarray(W1, dtype=np.float32)
    b1 = np.asarray(b1, dtype=np.float32)
    w2 = np.asarray(w2, dtype=np.float32)
    W3 = np.asarray(W3, dtype=np.float32)
    b3 = np.asarray(b3, dtype=np.float32)
    out, _ = run(hidden_state, encoder_outputs, W1, b1, w2, W3, b3, nb=4, j_tiles=8)
    return out


# revision 25
# speedup vs baseline: 1.0242x; 1.0011x over previous
"""Trainium2 Bass kernel for the aux-attention module.

reference (per batch b):
    inputs = concat([enc[b], broadcast(hs[b])], -1)          # (S, 4096)
    hidden = tanh(inputs @ W1.T + b1)                        # (S, 1024)
    e      = hidden @ w2.T                                   # (S,)
    alpha  = softmax(e)
    ctx    = alpha @ enc[b]                                  # (3072,)
    out[b] = ctx @ W3.T + b3                                 # (1024,)

Strategy: data-parallel over batch (4 batches/core x 8 cores), weights
replicated. PE matmuls in fp16 (fp32 PSUM). Softmax without max-subtraction:
w = exp(e - 4) unnormalized, 1/sum(w) folded into the final output scaling.

The hs@W1h + b1 per-batch bias rows (hb) are computed on HOST (tiny GEMV) and
shipped 128-replicated so the per-tile bias add runs on DVE (in-place into
PSUM) instead of costing PE outer-product matmuls. Tiles are processed
j-major (batch rotates fastest) so the four final ctx chains pipeline under
each other's matmul streams. The first two tiles' k-loops are interleaved so
early PE demand for W1 chunks (~6.3 MB) matches DMA supply.

Per core, per 128-row tile (single pass over enc, f-major layout from host):
  - hidden = tanh(enc_tile @ W1e.T + hb) : PE k-loop + DVE bias add + ACT
  - e column via one fused DVE multiply+accumulate against broadcast w2
  - w = exp(e-4) (ACT), column -> row via a tiny cross-partition DMA,
    broadcast across partitions (K=1 matmul outer product), then
    ctx_partial[f-chunk] = sum_s w[s]*enc[f, s] as a DVE multiply +
    per-chunk reduce on the same f-major tile already in SBUF. This chain is
    emitted one row-tile behind the matmul stream so the PE never waits.
Tail: inv_l via reduce+reciprocal, out = (ctxT @ W3.T) * inv_l + b3.
"""

import numpy as np

try:  # persistent compile cache: repeated runs skip the walrus compile
    import jax

    jax.config.update("jax_compilation_cache_dir", "/tmp/jax_neff_cache")
    jax.config.update("jax_persistent_cache_min_compile_time_secs", 1.0)
except Exception:
    pass

import concourse.bass as bass
import concourse.tile as tile
from concourse import mybir
from concourse.bass import ds
from concourse import bass_utils

# ---------------------------------------------------------------------------
# Walrus in this container caps sync waits per instruction (one; two for
# EventSemaphore). Tile's tail drain carries one wait per live semaphore and
# Tile occasionally leaks multi-wait instructions; split extras onto cheap
# carriers.
from concourse import tile as _tile_mod
from concourse import mybir as _mybir


def _patched_drain_and_barrier(self, tick_clock, wait_clock):
    nc = self.nc
    drain_inst = nc.sync.drain()
    wait_clock.add_sem_waits(
        drain_inst.ins, _tile_mod.ScopedClock({None: tick_clock.global_clock})
    )
    si = drain_inst.ins.sync_info
    waits = list(si.on_wait) if si is not None else []
    if len(waits) > 1:
        drain_inst.ins.sync_info = _mybir.SyncInfo(on_update=[], on_wait=waits[:1])
        for w in waits[1:]:
            extra = nc.sync.nop(nofuse=True, hint="drain_wait_split")
            extra.ins.sync_info = _mybir.SyncInfo(on_update=[], on_wait=[w])
    nc.all_engine_barrier()
    assert self.sems is not None
    popped = nc._tile_sem_poison_stack.pop()
    assert popped is self._sem_poison
    nc.clear_and_free_semaphores(list(self.sems.allocated().values()))
    nc.all_engine_barrier()


_tile_mod.TileContext._drain_and_barrier = _patched_drain_and_barrier


def _split_multiwaits(nc):
    for fn in nc.m.functions:
        for blk in fn.blocks:
            out, changed = [], False
            for inst in list(blk.instructions):
                si = inst.sync_info
                waits = list(si.on_wait) if si is not None else []
                cap = 2 if inst.opcode == "EventSemaphore" else 1
                if len(waits) > cap:
                    changed = True
                    for idx, w in enumerate(waits[:-cap]):
                        nop = _mybir.InstNoOp(
                            name=f"{inst.name}-wsplit{idx}", ins=[], outs=[]
                        )
                        nop.engine = inst.engine
                        nop.sync_info = _mybir.SyncInfo(on_update=[], on_wait=[w])
                        out.append(nop)
                    inst.sync_info = _mybir.SyncInfo(
                        on_update=list(si.on_update), on_wait=waits[-cap:]
                    )
                out.append(inst)
            if changed:
                blk.instructions = out


# ---------------------------------------------------------------------------

F16 = mybir.dt.float16
F32 = mybir.dt.float32

N_CORES = 8
B, S, DIM, F = 32, 1024, 1024, 3072  # F = enc feature dim; DIM = model dim
KF = F // 128  # 24 enc k-tiles
EXP_SHIFT = -4.0  # w = exp(e + EXP_SHIFT); e is O(1), shift keeps fp16 safe


def _bcast_free(ap, n, at=1):
    """Insert a step-0 (broadcast) free dim of size n at position `at`."""
    aps = list(ap.ap)
    aps.insert(at, [0, n])
    return bass.AP(tensor=ap.tensor, offset=ap.offset, ap=aps)


def _bcast_part(ap, p):
    """View a [1, n] row as a [1, p, n] replication via a step-0 middle dim
    (DMA source view; the dest AP's partition dim carries the fan-out)."""
    aps = list(ap.ap)
    assert aps[0][1] == 1
    return bass.AP(
        tensor=ap.tensor, offset=ap.offset, ap=[aps[0], [0, p]] + aps[1:]
    )


def build_bass(nb, j_tiles):
    """nb batches per core, j_tiles row-tiles of 128 per batch."""
    nj = nb * j_tiles
    nc = bass.Bass()
    encT = nc.declare_dram_parameter("encT", [nj, 128, KF, 128], F16, isOutput=False)
    w1t = nc.declare_dram_parameter("w1t", [128, KF, DIM], F16, isOutput=False)
    w3t = nc.declare_dram_parameter("w3t", [128, KF, DIM], F16, isOutput=False)
    hbb = nc.declare_dram_parameter("hbb", [128, nb, DIM], F16, isOutput=False)
    w2b = nc.declare_dram_parameter("w2b", [128, DIM], F16, isOutput=False)
    b3b = nc.declare_dram_parameter("b3b", [nb, DIM], F32, isOutput=False)
    onesb = nc.declare_dram_parameter("onesb", [1, 128], F16, isOutput=False)
    eyeb = nc.declare_dram_parameter("eyeb", [128, 128], F16, isOutput=False)
    out_d = nc.declare_dram_parameter("out", [nb, DIM], F32, isOutput=True)

    # tile t -> (b, j), j-major: the last nb tiles are the final row-tile of
    # each batch, so their ctx chains pipeline under each other's matmuls.
    seq = [(t % nb, t // nb) for t in range(nj)]
    # first tiles run interleaved k-loops so PE demand for each W1 chunk
    # tracks DMA supply; with 3 tiles the third lags by `lag` k-steps (its
    # PSUM group accumulates k=lag..KF-1 first, then wraps to k=0..lag-1)
    n_skew = 3 if nj >= 4 else min(2, nj)

    with tile.TileContext(nc) as tc:
        with (
            tc.tile_pool(name="consts", bufs=1) as consts,
            tc.tile_pool(name="encT", bufs=5) as encT_pool,
            tc.tile_pool(name="tanh", bufs=3) as tanh_pool,
            tc.tile_pool(name="scratch", bufs=1) as scratch_pool,
            tc.tile_pool(name="prod", bufs=2) as prod_pool,
            tc.tile_pool(name="wrow", bufs=3) as wrow_pool,
            tc.tile_pool(name="ctxa", bufs=2) as ctxa_pool,
            tc.tile_pool(name="ps", bufs=4, space="PSUM") as ps,
        ):
            # ---- resident constants ----
            # DMA emission order is the schedule priority: the PE needs et
            # tiles for the skew group plus W1 chunks in k order, everything
            # else after.
            w1t_sb = consts.tile([128, KF, DIM], F16)
            n_pre = min(4, nj)  # et tiles DMA'd during the w1t load
            ets = {}
            for t in range(n_pre):
                ets[t] = encT_pool.tile(
                    [128, KF, 128], F16, tag="et", name=f"et{t}"
                )
            hbb_sb = consts.tile([128, nb, DIM], F16)
            # batched issues: each dma_start costs ~0.6us of serial SP
            # issue time, so W1 chunks go in groups (descriptors inside one
            # issue already fan out across all 16 DMA engines); early et
            # tiles and hbb are interleaved so PE startup demand tracks
            # supply; et0's k=0 chunk goes first so the PE starts ASAP
            def w1_group(lo, hi):
                nc.sync.dma_start(out=w1t_sb[:, lo:hi, :], in_=w1t[:, lo:hi, :])

            nc.sync.dma_start(out=ets[0][:, 0:1, :], in_=encT[0][:, 0:1, :])
            w1_group(0, 2)
            nc.sync.dma_start(out=ets[0][:, 1:, :], in_=encT[0][:, 1:, :])
            if nj > 1:
                nc.sync.dma_start(out=ets[1], in_=encT[1])
            w1_group(2, 4)
            w1_group(4, 8)
            if nj > 2:
                nc.sync.dma_start(out=ets[2], in_=encT[2])
            w1_group(8, 12)
            w1_group(12, 16)
            nc.sync.dma_start(out=hbb_sb, in_=hbb[:])
            w1_group(16, 20)
            if nj > 3:
                nc.sync.dma_start(out=ets[3], in_=encT[3])
            w1_group(20, KF)
            w2b_sb = consts.tile([128, DIM], F16)
            nc.sync.dma_start(out=w2b_sb, in_=w2b[:])
            ones_sb = consts.tile([1, 128], F16)
            nc.sync.dma_start(out=ones_sb, in_=onesb[:])
            eye_sb = consts.tile([128, 128], F16)
            nc.sync.dma_start(out=eye_sb, in_=eyeb[:])
            # tail-only constants declared here, loaded late (low priority)
            w3t_sb = consts.tile([128, KF, DIM], F16)
            b3_sb = consts.tile([nb, DIM], F32)

            negc_sb = consts.tile([128, 1], F32)
            nc.vector.memset(negc_sb, EXP_SHIFT)

            e_sb = consts.tile([128, nj], F32)
            e2_sb = consts.tile([128, 2], F32)
            lparts_sb = consts.tile([1, nb, j_tiles], F32)
            linv_sb = consts.tile([1, nb], F32)
            invl_sb = consts.tile([nb, 1], F32)
            ctxT_sb = consts.tile([128, KF, nb], F16)
            out_sb = consts.tile([nb, DIM], F32)

            ctx_accs = {}
            pending = None
            p30 = wb30 = pr30 = None

            def emit_wb_bcast(wr):
                # broadcast w across partitions via K=1 outer product
                wbp = ps.tile([128, 128], F32, tag="wb", bufs=2)
                nc.tensor.matmul(wbp, ones_sb, wr, start=True, stop=True)
                wb = wrow_pool.tile([128, 128], F16, tag="wb")
                nc.vector.tensor_copy(wb, wbp)
                return wb

            def emit_ctx_half(state, wb, pr, lo, hi):
                # ctx_partial[f-chunk c] = sum_s wb[:, s] * et[:, c, s]
                b, j, et, wr = state
                ctx_acc = ctx_accs[b]
                nc.vector.tensor_mul(
                    pr[:, lo:hi, :], et[:, lo:hi, :], _bcast_free(wb[:], hi - lo)
                )
                cpart = ctxa_pool.tile(
                    [128, hi - lo], F32, tag="cpart", bufs=2, name=f"cp{lo}"
                )
                nc.vector.tensor_reduce(
                    out=cpart,
                    in_=pr[:, lo:hi, :],
                    axis=mybir.AxisListType.X,
                    op=mybir.AluOpType.add,
                )
                if j == 0:
                    nc.vector.tensor_copy(ctx_acc[:, lo:hi], cpart)
                else:
                    nc.vector.tensor_add(ctx_acc[:, lo:hi], ctx_acc[:, lo:hi], cpart)
                if j == j_tiles - 1:
                    # ctxT column for this batch (f16 for the W3 matmuls)
                    nc.vector.tensor_copy(ctxT_sb[:, lo:hi, b], ctx_acc[:, lo:hi])

            def emit_ctx_tail(state, split=False):
                # split=True: emit in graduated pieces so the W3 k-loop
                # starts consuming ctxT chunks while later pieces reduce
                wb = emit_wb_bcast(state[3])
                pr = prod_pool.tile([128, KF, 128], F16, name="pr")
                bounds = [0, 2, 6, 12, 18, KF] if split else [0, KF]
                for lo, hi in zip(bounds, bounds[1:]):
                    emit_ctx_half(state, wb, pr, lo, hi)

            def emit_post(t, et, hps):
                """bias add + tanh + e + exp + w-row for tile t."""
                nonlocal pending
                b, j = seq[t]
                th = tanh_pool.tile([128, DIM], F16)
                for nh in range(2):
                    sl = ds(nh * 512, 512)
                    nc.vector.tensor_add(hps[nh], hps[nh], hbb_sb[:, b, sl])
                    nc.scalar.activation(
                        th[:, sl], hps[nh], mybir.ActivationFunctionType.Tanh
                    )
                sc = scratch_pool.tile([128, DIM], F16)
                nc.vector.scalar_tensor_tensor(
                    out=sc,
                    in0=th,
                    scalar=1.0,
                    in1=w2b_sb,
                    op0=mybir.AluOpType.mult,
                    op1=mybir.AluOpType.mult,
                    accum_out=e_sb[:, t : t + 1],
                )
                # w = exp(e-4) as a column, spread to a row via DMA
                wc = wrow_pool.tile([128, 1], F16, tag="wc")
                nc.scalar.activation(
                    wc,
                    e_sb[:, t : t + 1],
                    mybir.ActivationFunctionType.Exp,
                    bias=negc_sb,
                )
                wr = wrow_pool.tile([1, 128], F16)
                nc.sync.dma_start(out=wr, in_=wc)
                nc.vector.tensor_reduce(
                    out=lparts_sb[0:1, b, j : j + 1],
                    in_=wr,
                    axis=mybir.AxisListType.X,
                    op=mybir.AluOpType.add,
                )
                if pending is not None:
                    emit_ctx_tail(pending)
                pending = (b, j, et, wr)

            for b in range(nb):
                ctx_accs[b] = ctxa_pool.tile(
                    [128, KF], F32, tag="ctx_acc", bufs=nb, name=f"ctx_acc{b}"
                )

            # ---- skew group: interleave k-loops of the first tiles so each
            # W1 chunk feeds 2*n_skew matmuls as it lands ----
            skew_ps = {
                (t, nh): ps.tile(
                    [128, 512], F32, tag="h", bufs=6, name=f"skew_ps{t}_{nh}"
                )
                for t in range(n_skew)
                for nh in range(2)
            }

            def skew_mm(t, nh, k, start, stop):
                nc.tensor.matmul(
                    skew_ps[(t, nh)],
                    ets[t][:, k, :],
                    w1t_sb[:, k, ds(nh * 512, 512)],
                    start=start,
                    stop=stop,
                )

            if n_skew == 3:
                lag = KF // 3
                for k in range(lag):
                    for t in range(2):
                        for nh in range(2):
                            skew_mm(t, nh, k, k == 0, False)
                for k in range(lag, KF):
                    for t in range(3):
                        for nh in range(2):
                            skew_mm(
                                t,
                                nh,
                                k,
                                (k == lag) if t == 2 else False,
                                (k == KF - 1) if t < 2 else False,
                            )
                for t in range(2):
                    emit_post(t, ets[t], [skew_ps[(t, 0)], skew_ps[(t, 1)]])
                for k in range(lag):
                    for nh in range(2):
                        skew_mm(2, nh, k, False, k == lag - 1)
                emit_post(2, ets[2], [skew_ps[(2, 0)], skew_ps[(2, 1)]])
            else:
                for k in range(KF):
                    for t in range(n_skew):
                        for nh in range(2):
                            skew_mm(t, nh, k, k == 0, k == KF - 1)
                for t in range(n_skew):
                    emit_post(t, ets[t], [skew_ps[(t, 0)], skew_ps[(t, 1)]])

            # ---- main loop ----
            w3_loaded = set()
            for t in range(n_skew, nj):
                if t in ets:
                    et = ets[t]
                else:
                    et = encT_pool.tile([128, KF, 128], F16, tag="et")
                    nc.sync.dma_start(out=et, in_=encT[t])
                # spread the w3t prefetch across the main loop in 4-chunk
                # groups (one SP issue each)
                if t >= min(4, nj - 1):
                    span = max(nj - min(4, nj - 1), 1)
                    pos = t - min(4, nj - 1)
                    lo, hi = pos * KF // span, (pos + 1) * KF // span
                    lo, hi = (lo + 3) // 4 * 4, (hi + 3) // 4 * 4
                    for kk in range(lo, min(hi, KF), 4):
                        nc.sync.dma_start(
                            out=w3t_sb[:, kk : kk + 4, :],
                            in_=w3t[:, kk : kk + 4, :],
                        )
                        w3_loaded.update(range(kk, kk + 4))
                hps = []
                for nh in range(2):
                    sl = ds(nh * 512, 512)
                    hp = ps.tile([128, 512], F32, tag="h", bufs=6)
                    hps.append(hp)
                    for k in range(KF):
                        nc.tensor.matmul(
                            hp,
                            et[:, k, :],
                            w1t_sb[:, k, sl],
                            start=(k == 0),
                            stop=(k == KF - 1),
                        )
                        if (
                            t == nj - 1
                            and nh == 0
                            and k == KF // 2
                            and pending is not None
                        ):
                            # flush half of the second-to-last tile's chain
                            # from mid k-loop (its wb input is long ready, no
                            # PE stall); the other half is emitted after this
                            # tile's own bias/e ops so those aren't queued
                            # behind 5us of DVE reduce work
                            p30 = pending
                            pending = None
                            wb30 = emit_wb_bcast(p30[3])
                            pr30 = prod_pool.tile([128, KF, 128], F16, name="pr30")
                            emit_ctx_half(p30, wb30, pr30, 0, KF // 2)
                    if t == nj - 1:
                        # pipelined last tile: each half's bias/tanh/e-dot
                        # runs under the other half's matmul stream
                        b, j = seq[t]
                        if nh == 0:
                            th_last = tanh_pool.tile([128, DIM], F16)
                            sc_last = scratch_pool.tile([128, DIM], F16)
                        nc.vector.tensor_add(hp, hp, hbb_sb[:, b, sl])
                        nc.scalar.activation(
                            th_last[:, sl], hp, mybir.ActivationFunctionType.Tanh
                        )
                        nc.vector.scalar_tensor_tensor(
                            out=sc_last[:, sl],
                            in0=th_last[:, sl],
                            scalar=1.0,
                            in1=w2b_sb[:, sl],
                            op0=mybir.AluOpType.mult,
                            op1=mybir.AluOpType.mult,
                            accum_out=e2_sb[:, nh : nh + 1],
                        )
                if t == nj - 1:
                    b, j = seq[t]
                    nc.vector.tensor_add(
                        e_sb[:, t : t + 1], e2_sb[:, 0:1], e2_sb[:, 1:2]
                    )
                    if p30 is not None:
                        emit_ctx_half(p30, wb30, pr30, KF // 2, KF)
                    # keep the PE clock hot through the serial exp/ctx
                    # window: redundant matmuls into a scratch psum (their
                    # inputs are resident, so they run during the idle gap)
                    for wk in range(16):
                        wp_warm = ps.tile(
                            [128, 512], F32, tag="h", bufs=6, name=f"warm{wk}"
                        )
                        nc.tensor.matmul(
                            wp_warm,
                            et[:, wk, :],
                            w1t_sb[:, wk, ds(0, 512)],
                            start=True,
                            stop=True,
                        )
                    wc = wrow_pool.tile([128, 1], F16, tag="wc")
                    nc.scalar.activation(
                        wc,
                        e_sb[:, t : t + 1],
                        mybir.ActivationFunctionType.Exp,
                        bias=negc_sb,
                    )
                    # column -> row via PE transpose (skips the ~2-3us
                    # SBUF-to-SBUF DMA round trip on the critical tail)
                    wtp = ps.tile([1, 128], F16, tag="wb", bufs=2)
                    nc.tensor.matmul(
                        wtp, wc, eye_sb, start=True, stop=True, is_transpose=True
                    )
                    wr = wrow_pool.tile([1, 128], F16)
                    nc.vector.tensor_copy(wr, wtp)
                    nc.vector.tensor_reduce(
                        out=lparts_sb[0:1, b, j : j + 1],
                        in_=wr,
                        axis=mybir.AxisListType.X,
                        op=mybir.AluOpType.add,
                    )
                    for wk in range(10):
                        wp_warm = ps.tile(
                            [128, 512], F32, tag="h", bufs=6, name=f"warmb{wk}"
                        )
                        nc.tensor.matmul(
                            wp_warm,
                            et[:, wk, :],
                            w1t_sb[:, wk, ds(512, 512)],
                            start=True,
                            stop=True,
                        )
                    pending = (b, j, et, wr)
                else:
                    emit_post(t, et, hps)
            for kk in range(KF):  # stragglers (nj small or uneven spread)
                if kk not in w3_loaded:
                    nc.sync.dma_start(out=w3t_sb[:, kk, :], in_=w3t[:, kk, :])
            nc.sync.dma_start(out=b3_sb, in_=b3b[:])
            if pending is not None:
                emit_ctx_tail(pending, split=True)

            # ---- 1/l per batch, spread to a partition-column ----
            nc.vector.tensor_reduce(
                out=linv_sb,
                in_=lparts_sb,
                axis=mybir.AxisListType.X,
                op=mybir.AluOpType.add,
            )
            nc.vector.reciprocal(linv_sb, linv_sb)
            nc.sync.dma_start(out=invl_sb, in_=linv_sb[0:1, :])

            # ---- out = (ctx @ W3.T) * inv_l + b3 ----
            for nh in range(2):
                sl = ds(nh * 512, 512)
                wp = ps.tile([nb, 512], F32, tag="h", bufs=6)
                for k in range(KF):
                    nc.tensor.matmul(
                        wp,
                        ctxT_sb[:, k, :],
                        w3t_sb[:, k, sl],
                        start=(k == 0),
                        stop=(k == KF - 1),
                    )
                nc.vector.scalar_tensor_tensor(
                    out=out_sb[:, sl],
                    in0=wp,
                    scalar=invl_sb,
                    in1=b3_sb[:, sl],
                    op0=mybir.AluOpType.mult,
                    op1=mybir.AluOpType.add,
                )
            nc.sync.dma_start(out=out_d[:], in_=out_sb)

    _split_multiwaits(nc)
    return nc


def make_in_maps(hidden_state, encoder_outputs, W1, b1, w2, W3, b3, nb, j_tiles):
    """Shard + lay out the full inputs for each core. Returns list of dicts."""
    f16, f32 = np.float16, np.float32
    nj = nb * j_tiles
    s_core = j_tiles * 128

    w1t = np.ascontiguousarray(
        W1.T[:F].reshape(KF, 128, DIM).transpose(1, 0, 2)
    ).astype(f16)
    w3t = np.ascontiguousarray(
        W3.T.reshape(KF, 128, DIM).transpose(1, 0, 2)
    ).astype(f16)
    w2b = np.ascontiguousarray(np.broadcast_to(w2.reshape(1, DIM), (128, DIM))).astype(
        f16
    )
    onesb = np.ones((1, 128), f16)
    eyeb = np.eye(128, dtype=f16)
    b3b_full = np.ascontiguousarray(
        np.broadcast_to(b3.reshape(1, DIM), (nb, DIM))
    ).astype(f32)
    # per-batch bias rows: hb = hs @ W1h.T + b1, host-computed (tiny GEMV)
    hb_full = (
        hidden_state.astype(f32) @ W1[:, F:].T.astype(f32) + b1.astype(f32)
    ).astype(f16)

    in_maps = []
    for i in range(N_CORES):
        bs = slice(i * nb, (i + 1) * nb)
        enc_c = encoder_outputs[bs, :s_core, :]  # (nb, s_core, F)
        e5 = enc_c.reshape(nb, j_tiles, 128, KF, 128)
        # j-major tile order: tile t = j*nb + b
        encT = np.ascontiguousarray(e5.transpose(1, 0, 4, 3, 2)).astype(f16)
        hbb = np.ascontiguousarray(
            np.broadcast_to(hb_full[bs][None, :, :], (128, nb, DIM))
        )
        in_maps.append(
            {
                "encT": encT.reshape(nj, 128, KF, 128),
                "w1t": w1t,
                "w3t": w3t,
                "hbb": hbb,
                "w2b": w2b,
                "b3b": b3b_full,
                "onesb": onesb,
                "eyeb": eyeb,
            }
        )
    return in_maps


_CACHE = {}


def run(hidden_state, encoder_outputs, W1, b1, w2, W3, b3, nb, j_tiles, trace=False):
    key = (nb, j_tiles)
    if key not in _CACHE:
        _CACHE[key] = build_bass(nb, j_tiles)
    nc = _CACHE[key]
    in_maps = make_in_maps(
        hidden_state, encoder_outputs, W1, b1, w2, W3, b3, nb, j_tiles
    )
    res = bass_utils.run_bass_kernel_spmd(
        nc, in_maps, list(range(N_CORES)), trace=trace
    )
    out = np.concatenate([res.results[i]["out"] for i in range(N_CORES)], axis=0)
    return out.astype(np.float32), res


def kernel(hidden_state, encoder_outputs, W1, b1, w2, W3, b3):
    hidden_state = np.asarray(hidden_state, dtype=np.float32)
    encoder_outputs = np.asarray(encoder_outputs, dtype=np.float32)
    W1 = np.asarray(W1, dtype=np.float32)
    b1 = np.asarray(b1, dtype=np.float32)
    w2 = np.asarray(w2, dtype=np.float32)
    W3 = np.asarray(W3, dtype=np.float32)
    b3 = np.asarray(b3, dtype=np.float32)
    out, _ = run(hidden_state, encoder_outputs, W1, b1, w2, W3, b3, nb=4, j_tiles=8)
    return out
